# revision 3
# baseline (speedup 1.0000x reference)
"""Trainium2 Bass kernel for a character-CNN word encoder.

Computation (per word of W=20 chars):
  x = emb[chars]                       # [W, E=64] -> [E, W]
  y_k = conv1d(x, w_k, 'same') + b_k   # k in {1,3,5}, H=256 channels
  m_k = max_t relu(y_k)                # [H]
  out = concat(m1, m3, m5) @ lw.T + lb # [H]

Strategy (pure data parallel over N = B*S = 8192 words, 1024 words/core).
The original dma_gather embedding serialized ~190us/core on the Q7
descriptor generator; this version removes it entirely:
  - Embedding lookup as a MATMUL: the host uploads a one-hot encoding of
    the padded char stream (2 passes of 128 vocab rows each, bf16), and
    X[0:64, tok] = table.T @ onehot accumulates over the two K=128 passes
    into PSUM, then DVE-copies to SBUF bf16. Rows 64:128 of X (the
    one-column-left shift that lets one K=128 conv matmul contract two
    taps) are made by per-tile SBUF->SBUF DMAs.
  - Convs: tap-pair matmuls, 12 per 16-word block into 2 PSUM groups of
    3 regions x 512 fp32 (one bank each); PE streams back-to-back at
    ~135ns per 320-col matmul with LDWEIGHTS overlapped.
  - Drain: ACT copies each PSUM group block [128,3,16,20] into a bf16
    buffer [128,3,128,20]; the max over the 20 positions is a DVE
    tensor_tensor(max) tree (2x mode, ~2x cheaper per element than the
    1x tensor_reduce), with all slice starts 4-byte aligned. Tree/relu
    ops of chunk c are spread across chunk c+1's DVE stream, A/B groups
    interleaved, so dependent levels hide the DVE pipe drain and never
    block the PSUM-freeing drains. The last chunk's tree is split in w
    halves to shorten the epilogue.
  - relu(bias + max) via DVE tensor_scalar (add + max-0 fused).
  - Linear layer: 6 K=128 matmuls + K=1 ones-row matmul adding lb; its
    PSUM tile shares a 2-buffer pool with the embed tiles (8 banks total
    with the conv groups).
"""

import numpy as np
import ml_dtypes

import concourse.bass as bass
import concourse.tile as tile
import concourse.mybir as mybir
from concourse import bacc
from concourse.bass_utils import run_bass_kernel_spmd

BF16 = ml_dtypes.bfloat16

# Problem shape (hardcoded per contest rules).
B, S, W = 64, 128, 20
VOCAB, E, H = 256, 64, 256
N_CORES = 8
NW = (B * S) // N_CORES       # words per core = 1024
WP = 22                       # word frame: [z z t0..t19]
PAD_TOK = 300                 # never matches vocab 0..255 -> all-zero one-hot col
CHUNK_W = 128                 # words per chunk
N_CHUNKS = NW // CHUNK_W      # 8
NB = 16                       # words per conv-matmul block
N_BLOCKS = CHUNK_W // NB      # 8
TOK_CHUNK = CHUNK_W * WP      # 2816 tokens per chunk
HALO = 8                      # extra halo cols per chunk
TOKC = TOK_CHUNK + HALO       # 2824 cols per chunk tile
NT = NW * WP                  # 22528
NTP = NT + 16                 # padded one-hot cols in DRAM

# Embedding tile split of a chunk's TOKC columns (PSUM bank = 512 fp32).
EMB_TILES = [512, 512, 512, 512, 512, TOKC - 5 * 512]   # last = 264

# Conv tap-pair matmul plan (identical to v1).
# (region, conv_k, half, first_tap d, j0 = d - pad + 2, start, stop)
# Regions: 0=c5h0 1=c5h1 2=c3h0 3=c3h1 4=c1h0 5=c1h1.
def _mm_plan():
    plan_a, plan_b = [], []
    for half in (0, 1):
        r = half
        taps = [(0, 0), (2, 2), (4, 4)]
        for i, (d, j0) in enumerate(taps):
            plan_a.append((r, 5, half, d, j0, i == 0, i == len(taps) - 1))
    for half, r, dst in ((0, 2, plan_a), (1, 3, plan_b)):
        taps = [(0, 1), (2, 3)]
        for i, (d, j0) in enumerate(taps):
            dst.append((r, 3, half, d, j0, i == 0, i == len(taps) - 1))
    for half in (0, 1):
        r = 4 + half
        plan_b.append((r, 1, half, 0, 2, True, True))
    return plan_a, plan_b

PLAN_A, PLAN_B = _mm_plan()
WBLOCKS = PLAN_A + PLAN_B

# lw column ranges per region (reference concat order: conv1, conv3, conv5).
LW_COLS = {0: (512, 640), 1: (640, 768), 2: (256, 384), 3: (384, 512),
           4: (0, 128), 5: (128, 256)}

# ---- engine-assignment knobs (tuned from traces) ----
# ACT drains every (group, block) PSUM region into a bf16 buffer (the only
# PSUM-slot gating path, ~1us latency inside the ~1.6us matmul window); DVE
# runs the max tree at its 2x tensor_tensor mode on SBUF plus X-copies and
# relu. Trees of chunk c-1 are spread between chunk c's blocks and the two
# groups' trees are interleaved so dependent levels hide the DVE pipe drain.
# The last N_DIRECT blocks per group are instead reduced directly from PSUM
# by DVE (knob to rebalance ACT vs DVE).
N_DIRECT = 0
N_TREE = N_BLOCKS - N_DIRECT


def _build_nc():
    f32 = mybir.dt.float32
    bf16 = mybir.dt.bfloat16
    ALU = mybir.AluOpType
    AF = mybir.ActivationFunctionType

    nc = bacc.Bacc("TRN2", target_bir_lowering=False, debug=False)

    oh0_d = nc.dram_tensor("oh0", [128, NTP], bf16, kind="ExternalInput").ap()
    oh1_d = nc.dram_tensor("oh1", [128, NTP], bf16, kind="ExternalInput").ap()
    tbl_d = nc.dram_tensor("tbl", [128, 128], bf16, kind="ExternalInput").ap()
    wconv_d = nc.dram_tensor("wconv", [128, 12 * 128], bf16, kind="ExternalInput").ap()
    lwt_d = nc.dram_tensor("lwt", [128, 6 * 256], bf16, kind="ExternalInput").ap()
    cbias_d = nc.dram_tensor("cbias", [128, 6], f32, kind="ExternalInput").ap()
    lbias_d = nc.dram_tensor("lbias", [1, 256], bf16, kind="ExternalInput").ap()
    out_d = nc.dram_tensor("out", [NW, H], f32, kind="ExternalOutput").ap()

    with tile.TileContext(nc) as tc:
        with (
            tc.tile_pool(name="consts", bufs=1) as cpool,
            tc.tile_pool(name="oh0", bufs=2) as oh0pool,
            tc.tile_pool(name="oh1", bufs=2) as oh1pool,
            tc.tile_pool(name="xx", bufs=3) as xxpool,
            tc.tile_pool(name="dbuf", bufs=4) as dpool,
            tc.tile_pool(name="mtile", bufs=2) as mpool,
            tc.tile_pool(name="osb", bufs=2) as opool,
            tc.tile_pool(name="psC", bufs=2, space="PSUM") as psc_pool,
            # shared pool: embed tiles use [0:64, 0:512]; the linear-layer
            # output uses [:, 0:256]. bufs=2 double-buffers the embed tiles.
            tc.tile_pool(name="psE", bufs=2, space="PSUM") as pse_pool,
        ):
            # --- constants ---
            tbl_sb = cpool.tile([128, 128], bf16, tag="tbl")
            nc.sync.dma_start(tbl_sb[:], tbl_d[:])
            wconv_sb = cpool.tile([128, 12 * 128], bf16, tag="wconv")
            nc.sync.dma_start(wconv_sb[:], wconv_d[:])
            lwt_sb = cpool.tile([128, 6 * 256], bf16, tag="lwt")
            nc.sync.dma_start(lwt_sb[:], lwt_d[:])
            cbias_sb = cpool.tile([128, 6], f32, tag="cbias")
            nc.sync.dma_start(cbias_sb[:], cbias_d[:])
            lbias_sb = cpool.tile([1, 256], bf16, tag="lbias")
            nc.sync.dma_start(lbias_sb[:], lbias_d[:])
            ones_sb = cpool.tile([1, 128], bf16, tag="ones")
            nc.vector.memset(ones_sb[:], 1.0)

            oh_tiles = {}     # c -> (oh0_t, oh1_t)
            xx_tiles = {}     # c -> xx tile
            dbufs = {}        # (c, g) -> drain buffer tile
            m_pres = {}       # c -> m_pre
            m_alls = {}       # c -> m_all

            def load_oh(c):
                oh0_t = oh0pool.tile([128, TOKC], bf16, tag="oh0")
                nc.sync.dma_start(oh0_t[:], oh0_d[:, c * TOK_CHUNK: c * TOK_CHUNK + TOKC])
                oh1_t = oh1pool.tile([128, TOKC], bf16, tag="oh1")
                nc.sync.dma_start(oh1_t[:], oh1_d[:, c * TOK_CHUNK: c * TOK_CHUNK + TOKC])
                oh_tiles[c] = (oh0_t, oh1_t)

            def embed_tile(c, t):
                # 2 K=128 matmul passes into ps_e, then copy to xx bf16.
                if t == 0:
                    xx_t = xxpool.tile([128, TOKC], bf16, tag="xx")
                    xx_tiles[c] = xx_t
                o0, o1 = oh_tiles[c]
                c0 = sum(EMB_TILES[:t])
                n = EMB_TILES[t]
                ps_e = pse_pool.tile([128, 512], f32, tag="psE")
                nc.tensor.matmul(ps_e[0:64, 0:n], lhsT=tbl_sb[:, 0:64],
                                 rhs=o0[:, c0:c0 + n], start=True, stop=False)
                nc.tensor.matmul(ps_e[0:64, 0:n], lhsT=tbl_sb[:, 64:128],
                                 rhs=o1[:, c0:c0 + n], start=False, stop=True)
                xx = xx_tiles[c]
                nc.vector.tensor_copy(out=xx[0:64, c0:c0 + n], in_=ps_e[0:64, 0:n])

            def shift_part(c, t):
                # shifted-rows DMA for just the column range of embed tile t,
                # so conv blocks unblock tile-by-tile instead of waiting for
                # the whole chunk's X.
                c0 = sum(EMB_TILES[:t])
                n = EMB_TILES[t]
                a = c0 - 1 if t > 0 else 0
                e = c0 + n - 1
                xx = xx_tiles[c]
                nc.sync.dma_start(xx[64:128, a:e], xx[0:64, a + 1:e + 1])

            def conv_block(c, b):
                xx = xx_tiles[c]
                base = b * NB * WP
                if b == 0:
                    dbuf_a = dpool.tile([128, 3, N_TREE * NB, W], bf16, tag="dbuf")
                    dbuf_b = dpool.tile([128, 3, N_TREE * NB, W], bf16, tag="dbuf")
                    dbufs[(c, 0)] = dbuf_a
                    dbufs[(c, 1)] = dbuf_b

                def run_mms(plan, ps):
                    for (r, _k, _h, _d, j0, start, stop) in plan:
                        q = WBLOCKS.index((r, _k, _h, _d, j0, start, stop))
                        slot = r % 3
                        rhs = (
                            xx[:, base + j0: base + j0 + NB * WP]
                            .rearrange("p (w c) -> p w c", c=WP)[:, :, 0:W]
                        )
                        nc.tensor.matmul(
                            ps[:, slot * 512: slot * 512 + NB * W],
                            lhsT=wconv_sb[:, q * 128:(q + 1) * 128],
                            rhs=rhs, start=start, stop=stop,
                        )

                def drain(g, ps):
                    pv = (
                        ps[:, 0:1536]
                        .rearrange("p (r s) -> p r s", s=512)[:, :, 0:NB * W]
                        .rearrange("p r (w c) -> p r w c", c=W)
                    )
                    if b < N_TREE:
                        dst = dbufs[(c, g)][:, :, b * NB:(b + 1) * NB, :]
                        nc.scalar.copy(out=dst, in_=pv)
                    else:
                        nc.vector.tensor_reduce(
                            out=m_pres[c][:, g * 3:(g + 1) * 3,
                                          b * NB:(b + 1) * NB],
                            in_=pv, axis=mybir.AxisListType.X,
                            op=mybir.AluOpType.max)

                ps_a = psc_pool.tile([128, 1536], f32, tag="psC")
                run_mms(PLAN_A, ps_a)
                ps_b = psc_pool.tile([128, 1536], f32, tag="psC")
                run_mms(PLAN_B, ps_b)
                drain(0, ps_a)
                drain(1, ps_b)

            def tree_ops(c, w0=0, w1=None):
                # Max over the W=20 position cols of both groups' dbufs,
                # writing m_pre rows, then relu+bias into m_all. Returned as
                # a list of thunks so the caller can spread them across the
                # next chunk's DVE stream; A/B-group levels are interleaved
                # so each op's dependency is 2 ops back (hides pipe drain).
                # All tensor_tensor slices keep 4-byte-aligned starts (2x).
                MAX = mybir.AluOpType.max
                ops = []
                if w1 is None:
                    w1 = N_TREE * NB

                def lvl(g, s0, s1, n):
                    d = dbufs[(c, g)][:, :, w0:w1, :]
                    ops.append(lambda d=d, s0=s0, s1=s1, n=n: nc.vector.tensor_tensor(
                        out=d[:, :, :, s0:s0 + n], in0=d[:, :, :, s0:s0 + n],
                        in1=d[:, :, :, s1:s1 + n], op=MAX))

                for g in (0, 1):
                    lvl(g, 0, 8, 8)      # P1: t[0:8]  = max(t[0:8],  t[8:16])
                for g in (0, 1):
                    lvl(g, 0, 4, 4)      # P2: t[0:4]  = max(t[0:4],  t[4:8])
                for g in (0, 1):
                    lvl(g, 16, 18, 2)    # P3: t[16:18] = max(t[16:18], t[18:20])
                for g in (0, 1):
                    lvl(g, 0, 2, 2)      # P4: t[0:2]  = max(t[0:2],  t[2:4])
                for g in (0, 1):
                    lvl(g, 0, 16, 2)     # P5: t[0:2]  = max(t[0:2],  t[16:18])
                for g in (0, 1):         # P6: m_pre rows = max(t[0], t[1])
                    def p6(g=g):
                        d = dbufs[(c, g)][:, :, w0:w1, :]
                        mp = m_pres[c][:, g * 3:(g + 1) * 3, w0:w1]
                        t0 = d[:, :, :, 0:1].rearrange("p r w c -> p r (w c)")
                        t1 = d[:, :, :, 1:2].rearrange("p r w c -> p r (w c)")
                        nc.vector.tensor_tensor(out=mp, in0=t0, in1=t1, op=MAX)
                    ops.append(p6)
                return ops

            def relu_ops(c):
                ops = []
                for r in range(6):       # relu(bias + m_pre) -> m_all
                    def rl(r=r):
                        nc.vector.tensor_scalar(
                            out=m_alls[c][:, r, :], in0=m_pres[c][:, r, :],
                            scalar1=cbias_sb[:, r:r + 1], scalar2=0.0,
                            op0=mybir.AluOpType.add, op1=mybir.AluOpType.max)
                    ops.append(rl)
                return ops

            def linear(c):
                ps_t = pse_pool.tile([128, 512], f32, tag="psE")
                op = ps_t[:, 0:256]
                for r in range(6):
                    nc.tensor.matmul(
                        op[:], lhsT=m_alls[c][:, r, :],
                        rhs=lwt_sb[:, r * 256:(r + 1) * 256],
                        start=(r == 0), stop=False,
                    )
                nc.tensor.matmul(op[:], lhsT=ones_sb[0:1, :], rhs=lbias_sb[0:1, :],
                                 start=False, stop=True)
                osb = opool.tile([128, 256], f32, tag="osb")
                nc.vector.tensor_copy(out=osb[:], in_=op[:])
                nc.sync.dma_start(out_d[c * CHUNK_W:(c + 1) * CHUNK_W, :], osb[:])

            # --- schedule ---
            load_oh(0)
            load_oh(1)

            # embed tiles needed (exclusive upper idx) before conv block b:
            # block b reads xx cols up to b*352+353 (+1 for the shift source).
            TILES_FOR_BLOCK = [1, 2, 3, 3, 4, 5, 5, 6]
            emb_done = {}

            def ensure_embed(c, upto):
                while emb_done.get(c, 0) < min(upto, len(EMB_TILES)):
                    t = emb_done.get(c, 0)
                    embed_tile(c, t)
                    shift_part(c, t)
                    emb_done[c] = t + 1

            pending = []  # DVE tree/relu thunks of the previous chunk
            for c in range(N_CHUNKS):
                if c + 2 < N_CHUNKS:
                    load_oh(c + 2)
                m_pre = mpool.tile([128, 6, CHUNK_W], bf16, tag="m_pre")
                m_all = mpool.tile([128, 6, CHUNK_W], bf16, tag="m_all")
                m_pres[c] = m_pre
                m_alls[c] = m_all
                for b in range(N_BLOCKS):
                    ensure_embed(c, TILES_FOR_BLOCK[b])
                    if c + 1 < N_CHUNKS:
                        ensure_embed(c + 1, b + 1)
                    conv_block(c, b)
                    # spread previous chunk's tree/relu ops (3 per block
                    # keeps them all emitted before linear(c-1) at b==6)
                    for _ in range(3):
                        if pending:
                            pending.pop(0)()
                    if c == N_CHUNKS - 1 and b >= 4:
                        # last chunk: its first half-tree (blocks 0-3 drained)
                        # runs inline to shorten the epilogue
                        for _ in range(3):
                            if pending:
                                pending.pop(0)()
                    if b == 6 and c >= 1:
                        linear(c - 1)
                    if c == N_CHUNKS - 1 and b == 3:
                        pending += tree_ops(c, 0, 4 * NB)
                while pending:
                    pending.pop(0)()
                if c < N_CHUNKS - 1:
                    pending = tree_ops(c) + relu_ops(c)
            for op_ in tree_ops(N_CHUNKS - 1, 4 * NB, 8 * NB) + relu_ops(N_CHUNKS - 1):
                op_()
            linear(N_CHUNKS - 1)

    nc.compile()
    return nc


def _prep_maps(chars, emb, w1, b1, w3, b3, w5, b5, lw, lb):
    flat = np.asarray(chars).reshape(-1, W).astype(np.int64)  # [8192, 20]
    emb = np.asarray(emb, dtype=np.float32)
    lw = np.asarray(lw, dtype=np.float32)
    convs = {1: np.asarray(w1, np.float32), 3: np.asarray(w3, np.float32),
             5: np.asarray(w5, np.float32)}
    biases = {1: np.asarray(b1, np.float32), 3: np.asarray(b3, np.float32),
              5: np.asarray(b5, np.float32)}

    # tbl: cols 0:64 = emb rows 0:128, cols 64:128 = emb rows 128:256.
    tbl = np.zeros((128, 128), dtype=BF16)
    tbl[:, 0:E] = emb[0:128, :].astype(BF16)
    tbl[:, 64:64 + E] = emb[128:256, :].astype(BF16)

    wconv = np.zeros((128, 12 * 128), dtype=BF16)
    for q, (r, k, half, d, j0, _s, _e) in enumerate(WBLOCKS):
        wk = convs[k]  # [H, E, k]
        blk = np.zeros((128, 128), dtype=np.float32)
        blk[:E, :] = wk[half * 128:(half + 1) * 128, :, d].T
        if d + 1 < k:
            blk[E:, :] = wk[half * 128:(half + 1) * 128, :, d + 1].T
        wconv[:, q * 128:(q + 1) * 128] = blk.astype(BF16)

    lwt = np.zeros((128, 6 * 256), dtype=BF16)
    for r in range(6):
        lo, hi = LW_COLS[r]
        lwt[:, r * 256:(r + 1) * 256] = lw[:, lo:hi].T.astype(BF16)

    cbias = np.zeros((128, 6), dtype=np.float32)
    for r, (k, half) in enumerate([(5, 0), (5, 1), (3, 0), (3, 1), (1, 0), (1, 1)]):
        cbias[:, r] = biases[k][half * 128:(half + 1) * 128]

    lbias = np.asarray(lb, np.float32).reshape(1, 256).astype(BF16)

    rlo = np.arange(128, dtype=np.int16)[:, None]
    in_maps = []
    for c in range(N_CORES):
        words = flat[c * NW:(c + 1) * NW]  # [NW, 20]
        padded = np.full((NW, WP), PAD_TOK, dtype=np.int16)
        padded[:, 2:2 + W] = words
        stream = np.full(NTP, PAD_TOK, dtype=np.int16)
        stream[:NT] = padded.reshape(-1)
        oh0 = (stream[None, :] == rlo).astype(BF16)
        oh1 = (stream[None, :] == (rlo + 128)).astype(BF16)
        in_maps.append({
            "oh0": oh0, "oh1": oh1, "tbl": tbl, "wconv": wconv, "lwt": lwt,
            "cbias": cbias, "lbias": lbias,
        })
    return in_maps


_NC_CACHE = {}


def run(inputs, trace=False):
    if "nc" not in _NC_CACHE:
        _NC_CACHE["nc"] = _build_nc()
    nc = _NC_CACHE["nc"]
    in_maps = _prep_maps(**inputs)
    res = run_bass_kernel_spmd(nc, in_maps, list(range(N_CORES)), trace=trace)
    out = np.concatenate([res.results[i]["out"] for i in range(N_CORES)], axis=0)
    return out.reshape(B, S, H).astype(np.float32), res


def kernel(**inputs):
    out, _ = run(inputs)
    return out


# revision 4
# speedup vs baseline: 1.0528x; 1.0528x over previous
"""Trainium2 Bass kernel for a character-CNN word encoder.

Computation (per word of W=20 chars):
  x = emb[chars]                       # [W, E=64] -> [E, W]
  y_k = conv1d(x, w_k, 'same') + b_k   # k in {1,3,5}, H=256 channels
  m_k = max_t relu(y_k)                # [H]
  out = concat(m1, m3, m5) @ lw.T + lb # [H]

Strategy (pure data parallel over N = B*S = 8192 words, 1024 words/core).
The original dma_gather embedding serialized ~190us/core on the Q7
descriptor generator; this version removes it entirely:
  - Embedding lookup as a MATMUL: the host uploads a one-hot encoding of
    the padded char stream (2 passes of 128 vocab rows each, bf16), and
    X[0:64, tok] = table.T @ onehot accumulates over the two K=128 passes
    into PSUM, then DVE-copies to SBUF bf16. Rows 64:128 of X (the
    one-column-left shift that lets one K=128 conv matmul contract two
    taps) are made by per-tile SBUF->SBUF DMAs.
  - Convs: tap-pair matmuls, 12 per 16-word block into 2 PSUM groups of
    3 regions x 512 fp32 (one bank each); PE streams back-to-back at
    ~135ns per 320-col matmul with LDWEIGHTS overlapped.
  - Drain: ACT copies each PSUM group block [128,3,16,20] into a bf16
    buffer [128,3,128,20]; the max over the 20 positions is a DVE
    tensor_tensor(max) tree (2x mode, ~2x cheaper per element than the
    1x tensor_reduce), with all slice starts 4-byte aligned. Tree/relu
    ops of chunk c are spread across chunk c+1's DVE stream, A/B groups
    interleaved, so dependent levels hide the DVE pipe drain and never
    block the PSUM-freeing drains. The last chunk's tree is split in w
    halves to shorten the epilogue.
  - relu(bias + max) via DVE tensor_scalar (add + max-0 fused).
  - Linear layer: 6 K=128 matmuls + K=1 ones-row matmul adding lb; its
    PSUM tile shares a 2-buffer pool with the embed tiles (8 banks total
    with the conv groups).
"""

import numpy as np
import ml_dtypes

import concourse.bass as bass
import concourse.tile as tile
import concourse.mybir as mybir
from concourse import bacc
from concourse.bass_utils import run_bass_kernel_spmd

BF16 = ml_dtypes.bfloat16

# Problem shape (hardcoded per contest rules).
B, S, W = 64, 128, 20
VOCAB, E, H = 256, 64, 256
N_CORES = 8
NW = (B * S) // N_CORES       # words per core = 1024
WP = 22                       # word frame: [z z t0..t19]
PAD_TOK = 300                 # never matches vocab 0..255 -> all-zero one-hot col
CHUNK_W = 128                 # words per chunk
N_CHUNKS = NW // CHUNK_W      # 8
# Conv-matmul block sizes per chunk: 24-word blocks keep each PSUM region
# (nb*20 = 480 fp32) inside one bank while cutting the drain count per chunk
# from 16 to 12 (ACT fixed overhead) and the stall boundaries from 8 to 6.
BLOCKS = [24, 24, 24, 24, 24, 8]
BLOCK_OFF = [0, 24, 48, 72, 96, 120]
N_BLOCKS = len(BLOCKS)        # 6
TOK_CHUNK = CHUNK_W * WP      # 2816 tokens per chunk
HALO = 8                      # extra halo cols per chunk
TOKC = TOK_CHUNK + HALO       # 2824 cols per chunk tile
NT = NW * WP                  # 22528
NTP = NT + 16                 # padded one-hot cols in DRAM

# Embedding tile split of a chunk's TOKC columns (PSUM bank = 512 fp32).
EMB_TILES = [512, 512, 512, 512, 512, TOKC - 5 * 512]   # last = 264

# Conv tap-pair matmul plan (identical to v1).
# (region, conv_k, half, first_tap d, j0 = d - pad + 2, start, stop)
# Regions: 0=c5h0 1=c5h1 2=c3h0 3=c3h1 4=c1h0 5=c1h1.
def _mm_plan():
    plan_a, plan_b = [], []
    for half in (0, 1):
        r = half
        taps = [(0, 0), (2, 2), (4, 4)]
        for i, (d, j0) in enumerate(taps):
            plan_a.append((r, 5, half, d, j0, i == 0, i == len(taps) - 1))
    for half, r, dst in ((0, 2, plan_a), (1, 3, plan_b)):
        taps = [(0, 1), (2, 3)]
        for i, (d, j0) in enumerate(taps):
            dst.append((r, 3, half, d, j0, i == 0, i == len(taps) - 1))
    for half in (0, 1):
        r = 4 + half
        plan_b.append((r, 1, half, 0, 2, True, True))
    return plan_a, plan_b

PLAN_A, PLAN_B = _mm_plan()
WBLOCKS = PLAN_A + PLAN_B

# lw column ranges per region (reference concat order: conv1, conv3, conv5).
LW_COLS = {0: (512, 640), 1: (640, 768), 2: (256, 384), 3: (384, 512),
           4: (0, 128), 5: (128, 256)}

# ---- engine assignment (tuned from traces) ----
# ACT drains every (group, block) PSUM region into a bf16 buffer (the only
# PSUM-slot gating path); DVE runs the max tree at its 2x tensor_tensor
# mode on SBUF plus X-copies, relu, and the output-stage copy. Trees of
# chunk c-1 are spread between chunk c's blocks and the two groups' trees
# are interleaved so dependent levels hide the DVE pipe drain.


def _build_nc():
    f32 = mybir.dt.float32
    bf16 = mybir.dt.bfloat16
    ALU = mybir.AluOpType
    AF = mybir.ActivationFunctionType

    nc = bacc.Bacc("TRN2", target_bir_lowering=False, debug=False)

    oh0_d = nc.dram_tensor("oh0", [128, NTP], bf16, kind="ExternalInput").ap()
    oh1_d = nc.dram_tensor("oh1", [128, NTP], bf16, kind="ExternalInput").ap()
    tbl_d = nc.dram_tensor("tbl", [128, 128], bf16, kind="ExternalInput").ap()
    wconv_d = nc.dram_tensor("wconv", [128, 12 * 128], bf16, kind="ExternalInput").ap()
    lwt_d = nc.dram_tensor("lwt", [128, 6 * 256], bf16, kind="ExternalInput").ap()
    cbias_d = nc.dram_tensor("cbias", [128, 6], f32, kind="ExternalInput").ap()
    lbias_d = nc.dram_tensor("lbias", [1, 256], bf16, kind="ExternalInput").ap()
    out_d = nc.dram_tensor("out", [NW, H], f32, kind="ExternalOutput").ap()

    with tile.TileContext(nc) as tc:
        with (
            tc.tile_pool(name="consts", bufs=1) as cpool,
            tc.tile_pool(name="oh0", bufs=2) as oh0pool,
            tc.tile_pool(name="oh1", bufs=2) as oh1pool,
            tc.tile_pool(name="xx", bufs=3) as xxpool,
            tc.tile_pool(name="dbuf", bufs=4) as dpool,
            tc.tile_pool(name="mtile", bufs=2) as mpool,
            tc.tile_pool(name="osb", bufs=2) as opool,
            tc.tile_pool(name="psC", bufs=2, space="PSUM") as psc_pool,
            # shared pool: embed tiles use [0:64, 0:512]; the linear-layer
            # output uses [:, 0:256]. bufs=2 double-buffers the embed tiles.
            tc.tile_pool(name="psE", bufs=2, space="PSUM") as pse_pool,
        ):
            # --- constants ---
            tbl_sb = cpool.tile([128, 128], bf16, tag="tbl")
            nc.sync.dma_start(tbl_sb[:], tbl_d[:])
            wconv_sb = cpool.tile([128, 12 * 128], bf16, tag="wconv")
            nc.sync.dma_start(wconv_sb[:], wconv_d[:])
            lwt_sb = cpool.tile([128, 6 * 256], bf16, tag="lwt")
            nc.sync.dma_start(lwt_sb[:], lwt_d[:])
            cbias_sb = cpool.tile([128, 6], f32, tag="cbias")
            nc.sync.dma_start(cbias_sb[:], cbias_d[:])
            lbias_sb = cpool.tile([1, 256], bf16, tag="lbias")
            nc.sync.dma_start(lbias_sb[:], lbias_d[:])
            ones_sb = cpool.tile([1, 128], bf16, tag="ones")
            nc.vector.memset(ones_sb[:], 1.0)

            oh_tiles = {}     # c -> (oh0_t, oh1_t)
            xx_tiles = {}     # c -> xx tile
            dbufs = {}        # (c, g) -> drain buffer tile
            m_pres = {}       # c -> m_pre
            m_alls = {}       # c -> m_all

            def load_oh(c):
                oh0_t = oh0pool.tile([128, TOKC], bf16, tag="oh0")
                nc.sync.dma_start(oh0_t[:], oh0_d[:, c * TOK_CHUNK: c * TOK_CHUNK + TOKC])
                oh1_t = oh1pool.tile([128, TOKC], bf16, tag="oh1")
                nc.sync.dma_start(oh1_t[:], oh1_d[:, c * TOK_CHUNK: c * TOK_CHUNK + TOKC])
                oh_tiles[c] = (oh0_t, oh1_t)

            def embed_tile(c, t):
                # 2 K=128 matmul passes into ps_e, then copy to xx bf16.
                if t == 0:
                    xx_t = xxpool.tile([128, TOKC], bf16, tag="xx")
                    xx_tiles[c] = xx_t
                o0, o1 = oh_tiles[c]
                c0 = sum(EMB_TILES[:t])
                n = EMB_TILES[t]
                ps_e = pse_pool.tile([128, 512], f32, tag="psE")
                nc.tensor.matmul(ps_e[0:64, 0:n], lhsT=tbl_sb[:, 0:64],
                                 rhs=o0[:, c0:c0 + n], start=True, stop=False)
                nc.tensor.matmul(ps_e[0:64, 0:n], lhsT=tbl_sb[:, 64:128],
                                 rhs=o1[:, c0:c0 + n], start=False, stop=True)
                xx = xx_tiles[c]
                nc.vector.tensor_copy(out=xx[0:64, c0:c0 + n], in_=ps_e[0:64, 0:n])

            def shift_part(c, t):
                # shifted-rows DMA for just the column range of embed tile t,
                # so conv blocks unblock tile-by-tile instead of waiting for
                # the whole chunk's X.
                c0 = sum(EMB_TILES[:t])
                n = EMB_TILES[t]
                a = c0 - 1 if t > 0 else 0
                e = c0 + n - 1
                xx = xx_tiles[c]
                nc.sync.dma_start(xx[64:128, a:e], xx[0:64, a + 1:e + 1])

            def conv_block(c, b):
                xx = xx_tiles[c]
                nb = BLOCKS[b]
                off = BLOCK_OFF[b]
                base = off * WP
                if b == 0:
                    dbuf_a = dpool.tile([128, 3, CHUNK_W, W], bf16, tag="dbuf")
                    dbuf_b = dpool.tile([128, 3, CHUNK_W, W], bf16, tag="dbuf")
                    dbufs[(c, 0)] = dbuf_a
                    dbufs[(c, 1)] = dbuf_b

                def run_mms(plan, ps):
                    for (r, _k, _h, _d, j0, start, stop) in plan:
                        q = WBLOCKS.index((r, _k, _h, _d, j0, start, stop))
                        slot = r % 3
                        rhs = (
                            xx[:, base + j0: base + j0 + nb * WP]
                            .rearrange("p (w c) -> p w c", c=WP)[:, :, 0:W]
                        )
                        nc.tensor.matmul(
                            ps[:, slot * 512: slot * 512 + nb * W],
                            lhsT=wconv_sb[:, q * 128:(q + 1) * 128],
                            rhs=rhs, start=start, stop=stop,
                        )

                def drain(g, ps):
                    pv = (
                        ps[:, 0:1536]
                        .rearrange("p (r s) -> p r s", s=512)[:, :, 0:nb * W]
                        .rearrange("p r (w c) -> p r w c", c=W)
                    )
                    dst = dbufs[(c, g)][:, :, off:off + nb, :]
                    nc.scalar.copy(out=dst, in_=pv)

                ps_a = psc_pool.tile([128, 1536], f32, tag="psC")
                run_mms(PLAN_A, ps_a)
                ps_b = psc_pool.tile([128, 1536], f32, tag="psC")
                run_mms(PLAN_B, ps_b)
                drain(0, ps_a)
                drain(1, ps_b)

            def tree_ops(c, w0=0, w1=None):
                # Max over the W=20 position cols of both groups' dbufs,
                # writing m_pre rows, then relu+bias into m_all. Returned as
                # a list of thunks so the caller can spread them across the
                # next chunk's DVE stream; A/B-group levels are interleaved
                # so each op's dependency is 2 ops back (hides pipe drain).
                # All tensor_tensor slices keep 4-byte-aligned starts (2x).
                MAX = mybir.AluOpType.max
                ops = []
                if w1 is None:
                    w1 = CHUNK_W

                def lvl(g, s0, s1, n):
                    d = dbufs[(c, g)][:, :, w0:w1, :]
                    ops.append(lambda d=d, s0=s0, s1=s1, n=n: nc.vector.tensor_tensor(
                        out=d[:, :, :, s0:s0 + n], in0=d[:, :, :, s0:s0 + n],
                        in1=d[:, :, :, s1:s1 + n], op=MAX))

                for g in (0, 1):
                    lvl(g, 0, 8, 8)      # P1: t[0:8]  = max(t[0:8],  t[8:16])
                for g in (0, 1):
                    lvl(g, 0, 4, 4)      # P2: t[0:4]  = max(t[0:4],  t[4:8])
                for g in (0, 1):
                    lvl(g, 16, 18, 2)    # P3: t[16:18] = max(t[16:18], t[18:20])
                for g in (0, 1):
                    lvl(g, 0, 2, 2)      # P4: t[0:2]  = max(t[0:2],  t[2:4])
                for g in (0, 1):
                    lvl(g, 0, 16, 2)     # P5: t[0:2]  = max(t[0:2],  t[16:18])
                for g in (0, 1):         # P6: m_pre rows = max(t[0], t[1])
                    def p6(g=g):
                        d = dbufs[(c, g)][:, :, w0:w1, :]
                        mp = m_pres[c][:, g * 3:(g + 1) * 3, w0:w1]
                        t0 = d[:, :, :, 0:1].rearrange("p r w c -> p r (w c)")
                        t1 = d[:, :, :, 1:2].rearrange("p r w c -> p r (w c)")
                        nc.vector.tensor_tensor(out=mp, in0=t0, in1=t1, op=MAX)
                    ops.append(p6)
                return ops

            def relu_ops(c):
                ops = []
                for r in range(6):       # relu(bias + m_pre) -> m_all
                    def rl(r=r):
                        nc.vector.tensor_scalar(
                            out=m_alls[c][:, r, :], in0=m_pres[c][:, r, :],
                            scalar1=cbias_sb[:, r:r + 1], scalar2=0.0,
                            op0=mybir.AluOpType.add, op1=mybir.AluOpType.max)
                    ops.append(rl)
                return ops

            def linear(c):
                ps_t = pse_pool.tile([128, 512], f32, tag="psE")
                op = ps_t[:, 0:256]
                for r in range(6):
                    nc.tensor.matmul(
                        op[:], lhsT=m_alls[c][:, r, :],
                        rhs=lwt_sb[:, r * 256:(r + 1) * 256],
                        start=(r == 0), stop=False,
                    )
                nc.tensor.matmul(op[:], lhsT=ones_sb[0:1, :], rhs=lbias_sb[0:1, :],
                                 start=False, stop=True)
                osb = opool.tile([128, 256], f32, tag="osb")
                nc.vector.tensor_copy(out=osb[:], in_=op[:])
                nc.sync.dma_start(out_d[c * CHUNK_W:(c + 1) * CHUNK_W, :], osb[:])

            # --- schedule ---
            load_oh(0)
            load_oh(1)

            # embed tiles needed (exclusive upper idx) before conv block b:
            # block b reads xx cols up to (off+nb-1)*22+23 (+1 shift source).
            TILES_FOR_BLOCK = [2, 3, 4, 5, 6, 6]
            emb_done = {}

            def ensure_embed(c, upto):
                while emb_done.get(c, 0) < min(upto, len(EMB_TILES)):
                    t = emb_done.get(c, 0)
                    embed_tile(c, t)
                    shift_part(c, t)
                    emb_done[c] = t + 1

            pending = []  # DVE tree/relu thunks of the previous chunk
            for c in range(N_CHUNKS):
                if c + 2 < N_CHUNKS:
                    load_oh(c + 2)
                m_pre = mpool.tile([128, 6, CHUNK_W], bf16, tag="m_pre")
                m_all = mpool.tile([128, 6, CHUNK_W], bf16, tag="m_all")
                m_pres[c] = m_pre
                m_alls[c] = m_all
                for b in range(N_BLOCKS):
                    ensure_embed(c, TILES_FOR_BLOCK[b])
                    if c + 1 < N_CHUNKS:
                        ensure_embed(c + 1, b + 1)
                    conv_block(c, b)
                    # spread previous chunk's tree/relu ops (4 per block
                    # keeps all 18 emitted before linear(c-1) at b==4)
                    for _ in range(4):
                        if pending:
                            pending.pop(0)()
                    if c == N_CHUNKS - 1 and b >= 3:
                        # last chunk: its first half-tree (blocks 0-2 drained)
                        # runs inline to shorten the epilogue
                        for _ in range(4):
                            if pending:
                                pending.pop(0)()
                    if b == 4 and c >= 1:
                        linear(c - 1)
                    if c == N_CHUNKS - 1 and b == 2:
                        pending += tree_ops(c, 0, 72)
                while pending:
                    pending.pop(0)()
                if c < N_CHUNKS - 1:
                    pending = tree_ops(c) + relu_ops(c)
            for op_ in tree_ops(N_CHUNKS - 1, 72, CHUNK_W) + relu_ops(N_CHUNKS - 1):
                op_()
            linear(N_CHUNKS - 1)

    nc.compile()
    return nc


def _prep_maps(chars, emb, w1, b1, w3, b3, w5, b5, lw, lb):
    flat = np.asarray(chars).reshape(-1, W).astype(np.int64)  # [8192, 20]
    emb = np.asarray(emb, dtype=np.float32)
    lw = np.asarray(lw, dtype=np.float32)
    convs = {1: np.asarray(w1, np.float32), 3: np.asarray(w3, np.float32),
             5: np.asarray(w5, np.float32)}
    biases = {1: np.asarray(b1, np.float32), 3: np.asarray(b3, np.float32),
              5: np.asarray(b5, np.float32)}

    # tbl: cols 0:64 = emb rows 0:128, cols 64:128 = emb rows 128:256.
    tbl = np.zeros((128, 128), dtype=BF16)
    tbl[:, 0:E] = emb[0:128, :].astype(BF16)
    tbl[:, 64:64 + E] = emb[128:256, :].astype(BF16)

    wconv = np.zeros((128, 12 * 128), dtype=BF16)
    for q, (r, k, half, d, j0, _s, _e) in enumerate(WBLOCKS):
        wk = convs[k]  # [H, E, k]
        blk = np.zeros((128, 128), dtype=np.float32)
        blk[:E, :] = wk[half * 128:(half + 1) * 128, :, d].T
        if d + 1 < k:
            blk[E:, :] = wk[half * 128:(half + 1) * 128, :, d + 1].T
        wconv[:, q * 128:(q + 1) * 128] = blk.astype(BF16)

    lwt = np.zeros((128, 6 * 256), dtype=BF16)
    for r in range(6):
        lo, hi = LW_COLS[r]
        lwt[:, r * 256:(r + 1) * 256] = lw[:, lo:hi].T.astype(BF16)

    cbias = np.zeros((128, 6), dtype=np.float32)
    for r, (k, half) in enumerate([(5, 0), (5, 1), (3, 0), (3, 1), (1, 0), (1, 1)]):
        cbias[:, r] = biases[k][half * 128:(half + 1) * 128]

    lbias = np.asarray(lb, np.float32).reshape(1, 256).astype(BF16)

    rlo = np.arange(128, dtype=np.int16)[:, None]
    in_maps = []
    for c in range(N_CORES):
        words = flat[c * NW:(c + 1) * NW]  # [NW, 20]
        padded = np.full((NW, WP), PAD_TOK, dtype=np.int16)
        padded[:, 2:2 + W] = words
        stream = np.full(NTP, PAD_TOK, dtype=np.int16)
        stream[:NT] = padded.reshape(-1)
        oh0 = (stream[None, :] == rlo).astype(BF16)
        oh1 = (stream[None, :] == (rlo + 128)).astype(BF16)
        in_maps.append({
            "oh0": oh0, "oh1": oh1, "tbl": tbl, "wconv": wconv, "lwt": lwt,
            "cbias": cbias, "lbias": lbias,
        })
    return in_maps


_NC_CACHE = {}


def run(inputs, trace=False):
    if "nc" not in _NC_CACHE:
        _NC_CACHE["nc"] = _build_nc()
    nc = _NC_CACHE["nc"]
    in_maps = _prep_maps(**inputs)
    res = run_bass_kernel_spmd(nc, in_maps, list(range(N_CORES)), trace=trace)
    out = np.concatenate([res.results[i]["out"] for i in range(N_CORES)], axis=0)
    return out.reshape(B, S, H).astype(np.float32), res


def kernel(**inputs):
    out, _ = run(inputs)
    return out


# revision 5
# speedup vs baseline: 1.1271x; 1.0706x over previous
"""Trainium2 Bass kernel for a character-CNN word encoder.

Computation (per word of W=20 chars):
  x = emb[chars]                       # [W, E=64] -> [E, W]
  y_k = conv1d(x, w_k, 'same') + b_k   # k in {1,3,5}, H=256 channels
  m_k = max_t relu(y_k)                # [H]
  out = concat(m1, m3, m5) @ lw.T + lb # [H]

Strategy (pure data parallel over N = B*S = 8192 words, 1024 words/core).
The original dma_gather embedding serialized ~190us/core on the Q7
descriptor generator; this version removes it entirely:
  - Embedding lookup as a MATMUL: the host uploads a one-hot encoding of
    the padded char stream (2 passes of 128 vocab rows each, bf16), and
    X[0:64, tok] = table.T @ onehot accumulates over the two K=128 passes
    into PSUM, then DVE-copies to SBUF bf16. Rows 64:128 of X (the
    one-column-left shift that lets one K=128 conv matmul contract two
    taps) are made by per-tile SBUF->SBUF DMAs.
  - Convs: tap-pair matmuls, 12 per 16-word block into 2 PSUM groups of
    3 regions x 512 fp32 (one bank each); PE streams back-to-back at
    ~135ns per 320-col matmul with LDWEIGHTS overlapped.
  - Drain: ACT copies each PSUM group block [128,3,16,20] into a bf16
    buffer [128,3,128,20]; the max over the 20 positions is a DVE
    tensor_tensor(max) tree (2x mode, ~2x cheaper per element than the
    1x tensor_reduce), with all slice starts 4-byte aligned. Tree/relu
    ops of chunk c are spread across chunk c+1's DVE stream, A/B groups
    interleaved, so dependent levels hide the DVE pipe drain and never
    block the PSUM-freeing drains. The last chunk's tree is split in w
    halves to shorten the epilogue.
  - relu(bias + max) via DVE tensor_scalar (add + max-0 fused).
  - Linear layer: 6 K=128 matmuls + K=1 ones-row matmul adding lb; its
    PSUM tile shares a 2-buffer pool with the embed tiles (8 banks total
    with the conv groups).
"""

import numpy as np
import ml_dtypes

import concourse.bass as bass
import concourse.tile as tile
import concourse.mybir as mybir
from concourse import bacc
from concourse.bass_utils import run_bass_kernel_spmd

BF16 = ml_dtypes.bfloat16
FP8 = ml_dtypes.float8_e4m3

# Problem shape (hardcoded per contest rules).
B, S, W = 64, 128, 20
VOCAB, E, H = 256, 64, 256
N_CORES = 8
NW = (B * S) // N_CORES       # words per core = 1024
WP = 22                       # word frame: [z z t0..t19]
PAD_TOK = 300                 # never matches vocab 0..255 -> all-zero one-hot col
CHUNK_W = 128                 # words per chunk
N_CHUNKS = NW // CHUNK_W      # 8
# Conv-matmul block sizes per chunk: 24-word blocks keep each PSUM region
# (nb*20 = 480 fp32) inside one bank while cutting the drain count per chunk
# from 16 to 12 (ACT fixed overhead) and the stall boundaries from 8 to 6.
BLOCKS = [24, 24, 24, 24, 24, 8]
BLOCK_OFF = [0, 24, 48, 72, 96, 120]
N_BLOCKS = len(BLOCKS)        # 6
TOK_CHUNK = CHUNK_W * WP      # 2816 tokens per chunk
HALO = 8                      # extra halo cols per chunk
TOKC = TOK_CHUNK + HALO       # 2824 cols per chunk tile
NT = NW * WP                  # 22528
NTP = NT + 16                 # padded one-hot cols in DRAM

# Embedding tile split of a chunk's TOKC columns (PSUM bank = 512 fp32).
EMB_TILES = [512, 512, 512, 512, 512, TOKC - 5 * 512]   # last = 264

# Conv tap-pair matmul plan (identical to v1).
# (region, conv_k, half, first_tap d, j0 = d - pad + 2, start, stop)
# Regions: 0=c5h0 1=c5h1 2=c3h0 3=c3h1 4=c1h0 5=c1h1.
def _mm_plan():
    plan_a, plan_b = [], []
    for half in (0, 1):
        r = half
        taps = [(0, 0), (2, 2), (4, 4)]
        for i, (d, j0) in enumerate(taps):
            plan_a.append((r, 5, half, d, j0, i == 0, i == len(taps) - 1))
    for half, r, dst in ((0, 2, plan_a), (1, 3, plan_b)):
        taps = [(0, 1), (2, 3)]
        for i, (d, j0) in enumerate(taps):
            dst.append((r, 3, half, d, j0, i == 0, i == len(taps) - 1))
    for half in (0, 1):
        r = 4 + half
        plan_b.append((r, 1, half, 0, 2, True, True))
    return plan_a, plan_b

PLAN_A, PLAN_B = _mm_plan()
WBLOCKS = PLAN_A + PLAN_B

# lw column ranges per region (reference concat order: conv1, conv3, conv5).
LW_COLS = {0: (512, 640), 1: (640, 768), 2: (256, 384), 3: (384, 512),
           4: (0, 128), 5: (128, 256)}

# ---- engine assignment (tuned from traces) ----
# ACT drains every (group, block) PSUM region into a bf16 buffer (the only
# PSUM-slot gating path); DVE runs the max tree at its 2x tensor_tensor
# mode on SBUF plus X-copies, relu, and the output-stage copy. Trees of
# chunk c-1 are spread between chunk c's blocks and the two groups' trees
# are interleaved so dependent levels hide the DVE pipe drain.


def _build_nc():
    f32 = mybir.dt.float32
    bf16 = mybir.dt.bfloat16
    ALU = mybir.AluOpType
    AF = mybir.ActivationFunctionType

    nc = bacc.Bacc("TRN2", target_bir_lowering=False, debug=False)

    fp8 = mybir.dt.float8e4
    # one-hot of the char stream as two fp8 k-tiles (vocab 0:128 / 128:256)
    oh_d = nc.dram_tensor("oh", [128, 2, NTP], fp8, kind="ExternalInput").ap()
    # embedding table split hi/lo in fp8: [:,0:2,:] = hi k-tiles, [:,2:4,:] =
    # lo k-tiles (lo = emb - fp8(emb), so X = hi@oh + lo@oh has ~0.4% error)
    tbl_d = nc.dram_tensor("tbl", [128, 4, 64], fp8, kind="ExternalInput").ap()
    wconv_d = nc.dram_tensor("wconv", [128, 12 * 128], bf16, kind="ExternalInput").ap()
    lwt_d = nc.dram_tensor("lwt", [128, 6 * 256], bf16, kind="ExternalInput").ap()
    cbias_d = nc.dram_tensor("cbias", [128, 6], f32, kind="ExternalInput").ap()
    lbias_d = nc.dram_tensor("lbias", [1, 256], bf16, kind="ExternalInput").ap()
    out_d = nc.dram_tensor("out", [NW, H], f32, kind="ExternalOutput").ap()

    with tile.TileContext(nc) as tc:
        with (
            tc.tile_pool(name="consts", bufs=1) as cpool,
            tc.tile_pool(name="oh", bufs=2) as ohpool,
            tc.tile_pool(name="xx", bufs=3) as xxpool,
            tc.tile_pool(name="dbuf", bufs=4) as dpool,
            tc.tile_pool(name="mtile", bufs=2) as mpool,
            tc.tile_pool(name="osb", bufs=2) as opool,
            tc.tile_pool(name="psC", bufs=2, space="PSUM") as psc_pool,
            # shared pool: embed tiles use [0:64, 0:512]; the linear-layer
            # output uses [:, 0:256]. bufs=2 double-buffers the embed tiles.
            tc.tile_pool(name="psE", bufs=2, space="PSUM") as pse_pool,
        ):
            # --- constants ---
            tbl_sb = cpool.tile([128, 4, 64], fp8, tag="tbl")
            nc.sync.dma_start(tbl_sb[:], tbl_d[:])
            wconv_sb = cpool.tile([128, 12 * 128], bf16, tag="wconv")
            nc.sync.dma_start(wconv_sb[:], wconv_d[:])
            lwt_sb = cpool.tile([128, 6 * 256], bf16, tag="lwt")
            nc.sync.dma_start(lwt_sb[:], lwt_d[:])
            cbias_sb = cpool.tile([128, 6], f32, tag="cbias")
            nc.sync.dma_start(cbias_sb[:], cbias_d[:])
            lbias_sb = cpool.tile([1, 256], bf16, tag="lbias")
            nc.sync.dma_start(lbias_sb[:], lbias_d[:])
            ones_sb = cpool.tile([1, 128], bf16, tag="ones")
            nc.vector.memset(ones_sb[:], 1.0)

            oh_tiles = {}     # c -> (oh0_t, oh1_t)
            xx_tiles = {}     # c -> xx tile
            dbufs = {}        # (c, g) -> drain buffer tile
            m_pres = {}       # c -> m_pre
            m_alls = {}       # c -> m_all

            def load_oh(c):
                oh_t = ohpool.tile([128, 2, TOKC], fp8, tag="oh")
                nc.sync.dma_start(oh_t[:], oh_d[:, :, c * TOK_CHUNK: c * TOK_CHUNK + TOKC])
                oh_tiles[c] = oh_t

            def embed_tile(c, t):
                # 2 fp8 DoubleRow matmuls (hi + lo correction, each K=256 via
                # two k-tiles at 0.5 cyc/col) into ps_e, then copy to xx bf16.
                if t == 0:
                    xx_t = xxpool.tile([128, TOKC], bf16, tag="xx")
                    xx_tiles[c] = xx_t
                oh_t = oh_tiles[c]
                c0 = sum(EMB_TILES[:t])
                n = EMB_TILES[t]
                ps_e = pse_pool.tile([128, 512], f32, tag="psE")
                nc.tensor.matmul(ps_e[0:64, 0:n], lhsT=tbl_sb[:, 0:2, :],
                                 rhs=oh_t[:, :, c0:c0 + n], start=True, stop=False,
                                 perf_mode=mybir.MatmulPerfMode.DoubleRow)
                nc.tensor.matmul(ps_e[0:64, 0:n], lhsT=tbl_sb[:, 2:4, :],
                                 rhs=oh_t[:, :, c0:c0 + n], start=False, stop=True,
                                 perf_mode=mybir.MatmulPerfMode.DoubleRow)
                xx = xx_tiles[c]
                nc.vector.tensor_copy(out=xx[0:64, c0:c0 + n], in_=ps_e[0:64, 0:n])

            def shift_part(c, t):
                # shifted-rows DMA for just the column range of embed tile t,
                # so conv blocks unblock tile-by-tile instead of waiting for
                # the whole chunk's X.
                c0 = sum(EMB_TILES[:t])
                n = EMB_TILES[t]
                a = c0 - 1 if t > 0 else 0
                e = c0 + n - 1
                xx = xx_tiles[c]
                nc.sync.dma_start(xx[64:128, a:e], xx[0:64, a + 1:e + 1])

            def conv_block(c, b):
                xx = xx_tiles[c]
                nb = BLOCKS[b]
                off = BLOCK_OFF[b]
                base = off * WP
                if b == 0:
                    dbuf_a = dpool.tile([128, 3, CHUNK_W, W], bf16, tag="dbuf")
                    dbuf_b = dpool.tile([128, 3, CHUNK_W, W], bf16, tag="dbuf")
                    dbufs[(c, 0)] = dbuf_a
                    dbufs[(c, 1)] = dbuf_b

                def run_mms(plan, ps):
                    for (r, _k, _h, _d, j0, start, stop) in plan:
                        q = WBLOCKS.index((r, _k, _h, _d, j0, start, stop))
                        slot = r % 3
                        rhs = (
                            xx[:, base + j0: base + j0 + nb * WP]
                            .rearrange("p (w c) -> p w c", c=WP)[:, :, 0:W]
                        )
                        nc.tensor.matmul(
                            ps[:, slot * 512: slot * 512 + nb * W],
                            lhsT=wconv_sb[:, q * 128:(q + 1) * 128],
                            rhs=rhs, start=start, stop=stop,
                        )

                def drain(g, ps):
                    pv = (
                        ps[:, 0:1536]
                        .rearrange("p (r s) -> p r s", s=512)[:, :, 0:nb * W]
                        .rearrange("p r (w c) -> p r w c", c=W)
                    )
                    dst = dbufs[(c, g)][:, :, off:off + nb, :]
                    nc.scalar.copy(out=dst, in_=pv)

                ps_a = psc_pool.tile([128, 1536], f32, tag="psC")
                run_mms(PLAN_A, ps_a)
                ps_b = psc_pool.tile([128, 1536], f32, tag="psC")
                run_mms(PLAN_B, ps_b)
                drain(0, ps_a)
                drain(1, ps_b)

            def tree_ops(c, w0=0, w1=None):
                # Max over the W=20 position cols of both groups' dbufs,
                # writing m_pre rows, then relu+bias into m_all. Returned as
                # a list of thunks so the caller can spread them across the
                # next chunk's DVE stream; A/B-group levels are interleaved
                # so each op's dependency is 2 ops back (hides pipe drain).
                # All tensor_tensor slices keep 4-byte-aligned starts (2x).
                MAX = mybir.AluOpType.max
                ops = []
                if w1 is None:
                    w1 = CHUNK_W

                def lvl(g, s0, s1, n):
                    d = dbufs[(c, g)][:, :, w0:w1, :]
                    ops.append(lambda d=d, s0=s0, s1=s1, n=n: nc.vector.tensor_tensor(
                        out=d[:, :, :, s0:s0 + n], in0=d[:, :, :, s0:s0 + n],
                        in1=d[:, :, :, s1:s1 + n], op=MAX))

                for g in (0, 1):
                    lvl(g, 0, 8, 8)      # P1: t[0:8]  = max(t[0:8],  t[8:16])
                for g in (0, 1):
                    lvl(g, 0, 4, 4)      # P2: t[0:4]  = max(t[0:4],  t[4:8])
                for g in (0, 1):
                    lvl(g, 16, 18, 2)    # P3: t[16:18] = max(t[16:18], t[18:20])
                for g in (0, 1):
                    lvl(g, 0, 2, 2)      # P4: t[0:2]  = max(t[0:2],  t[2:4])
                for g in (0, 1):
                    lvl(g, 0, 16, 2)     # P5: t[0:2]  = max(t[0:2],  t[16:18])
                for g in (0, 1):         # P6: m_pre rows = max(t[0], t[1])
                    def p6(g=g):
                        d = dbufs[(c, g)][:, :, w0:w1, :]
                        mp = m_pres[c][:, g * 3:(g + 1) * 3, w0:w1]
                        t0 = d[:, :, :, 0:1].rearrange("p r w c -> p r (w c)")
                        t1 = d[:, :, :, 1:2].rearrange("p r w c -> p r (w c)")
                        nc.vector.tensor_tensor(out=mp, in0=t0, in1=t1, op=MAX)
                    ops.append(p6)
                return ops

            def relu_ops(c):
                ops = []
                for r in range(6):       # relu(bias + m_pre) -> m_all
                    def rl(r=r):
                        nc.vector.tensor_scalar(
                            out=m_alls[c][:, r, :], in0=m_pres[c][:, r, :],
                            scalar1=cbias_sb[:, r:r + 1], scalar2=0.0,
                            op0=mybir.AluOpType.add, op1=mybir.AluOpType.max)
                    ops.append(rl)
                return ops

            def linear(c):
                ps_t = pse_pool.tile([128, 512], f32, tag="psE")
                op = ps_t[:, 0:256]
                for r in range(6):
                    nc.tensor.matmul(
                        op[:], lhsT=m_alls[c][:, r, :],
                        rhs=lwt_sb[:, r * 256:(r + 1) * 256],
                        start=(r == 0), stop=False,
                    )
                nc.tensor.matmul(op[:], lhsT=ones_sb[0:1, :], rhs=lbias_sb[0:1, :],
                                 start=False, stop=True)
                osb = opool.tile([128, 256], f32, tag="osb")
                nc.vector.tensor_copy(out=osb[:], in_=op[:])
                nc.sync.dma_start(out_d[c * CHUNK_W:(c + 1) * CHUNK_W, :], osb[:])

            # --- schedule ---
            load_oh(0)
            load_oh(1)

            # embed tiles needed (exclusive upper idx) before conv block b:
            # block b reads xx cols up to (off+nb-1)*22+23 (+1 shift source).
            TILES_FOR_BLOCK = [2, 3, 4, 5, 6, 6]
            emb_done = {}

            def ensure_embed(c, upto):
                while emb_done.get(c, 0) < min(upto, len(EMB_TILES)):
                    t = emb_done.get(c, 0)
                    embed_tile(c, t)
                    shift_part(c, t)
                    emb_done[c] = t + 1

            pending = []  # DVE tree/relu thunks of the previous chunk
            for c in range(N_CHUNKS):
                if c + 2 < N_CHUNKS:
                    load_oh(c + 2)
                m_pre = mpool.tile([128, 6, CHUNK_W], bf16, tag="m_pre")
                m_all = mpool.tile([128, 6, CHUNK_W], bf16, tag="m_all")
                m_pres[c] = m_pre
                m_alls[c] = m_all
                for b in range(N_BLOCKS):
                    ensure_embed(c, TILES_FOR_BLOCK[b])
                    if c + 1 < N_CHUNKS:
                        ensure_embed(c + 1, b + 1)
                    conv_block(c, b)
                    # spread previous chunk's tree/relu ops (4 per block
                    # keeps all 18 emitted before linear(c-1) at b==4)
                    for _ in range(4):
                        if pending:
                            pending.pop(0)()
                    if c == N_CHUNKS - 1 and b >= 3:
                        # last chunk: its first half-tree (blocks 0-2 drained)
                        # runs inline to shorten the epilogue
                        for _ in range(4):
                            if pending:
                                pending.pop(0)()
                    if b == 4 and c >= 1:
                        linear(c - 1)
                    if c == N_CHUNKS - 1 and b == 2:
                        pending += tree_ops(c, 0, 72)
                while pending:
                    pending.pop(0)()
                if c < N_CHUNKS - 1:
                    pending = tree_ops(c) + relu_ops(c)
            for op_ in tree_ops(N_CHUNKS - 1, 72, CHUNK_W) + relu_ops(N_CHUNKS - 1):
                op_()
            linear(N_CHUNKS - 1)

    nc.compile()
    return nc


def _prep_maps(chars, emb, w1, b1, w3, b3, w5, b5, lw, lb):
    flat = np.asarray(chars).reshape(-1, W).astype(np.int64)  # [8192, 20]
    emb = np.asarray(emb, dtype=np.float32)
    lw = np.asarray(lw, dtype=np.float32)
    convs = {1: np.asarray(w1, np.float32), 3: np.asarray(w3, np.float32),
             5: np.asarray(w5, np.float32)}
    biases = {1: np.asarray(b1, np.float32), 3: np.asarray(b3, np.float32),
              5: np.asarray(b5, np.float32)}

    # tbl: fp8 hi/lo split, k-tile layout [128, 4, 64]:
    # [:,0,:]=hi vocab 0:128, [:,1,:]=hi vocab 128:256, [:,2:4,:]=lo parts.
    hi = emb.astype(FP8)
    lo = (emb - hi.astype(np.float32)).astype(FP8)
    tbl = np.zeros((128, 4, 64), dtype=FP8)
    tbl[:, 0, :] = hi[0:128, :]
    tbl[:, 1, :] = hi[128:256, :]
    tbl[:, 2, :] = lo[0:128, :]
    tbl[:, 3, :] = lo[128:256, :]

    wconv = np.zeros((128, 12 * 128), dtype=BF16)
    for q, (r, k, half, d, j0, _s, _e) in enumerate(WBLOCKS):
        wk = convs[k]  # [H, E, k]
        blk = np.zeros((128, 128), dtype=np.float32)
        blk[:E, :] = wk[half * 128:(half + 1) * 128, :, d].T
        if d + 1 < k:
            blk[E:, :] = wk[half * 128:(half + 1) * 128, :, d + 1].T
        wconv[:, q * 128:(q + 1) * 128] = blk.astype(BF16)

    lwt = np.zeros((128, 6 * 256), dtype=BF16)
    for r in range(6):
        lo, hi = LW_COLS[r]
        lwt[:, r * 256:(r + 1) * 256] = lw[:, lo:hi].T.astype(BF16)

    cbias = np.zeros((128, 6), dtype=np.float32)
    for r, (k, half) in enumerate([(5, 0), (5, 1), (3, 0), (3, 1), (1, 0), (1, 1)]):
        cbias[:, r] = biases[k][half * 128:(half + 1) * 128]

    lbias = np.asarray(lb, np.float32).reshape(1, 256).astype(BF16)

    rlo = np.arange(128, dtype=np.int16)[:, None]
    in_maps = []
    for c in range(N_CORES):
        words = flat[c * NW:(c + 1) * NW]  # [NW, 20]
        padded = np.full((NW, WP), PAD_TOK, dtype=np.int16)
        padded[:, 2:2 + W] = words
        stream = np.full(NTP, PAD_TOK, dtype=np.int16)
        stream[:NT] = padded.reshape(-1)
        oh = np.zeros((128, 2, NTP), dtype=FP8)
        oh[:, 0, :] = (stream[None, :] == rlo)
        oh[:, 1, :] = (stream[None, :] == (rlo + 128))
        in_maps.append({
            "oh": oh, "tbl": tbl, "wconv": wconv, "lwt": lwt,
            "cbias": cbias, "lbias": lbias,
        })
    return in_maps


_NC_CACHE = {}


def run(inputs, trace=False):
    if "nc" not in _NC_CACHE:
        _NC_CACHE["nc"] = _build_nc()
    nc = _NC_CACHE["nc"]
    in_maps = _prep_maps(**inputs)
    res = run_bass_kernel_spmd(nc, in_maps, list(range(N_CORES)), trace=trace)
    out = np.concatenate([res.results[i]["out"] for i in range(N_CORES)], axis=0)
    return out.reshape(B, S, H).astype(np.float32), res


def kernel(**inputs):
    out, _ = run(inputs)
    return out


# revision 6
# speedup vs baseline: 1.1727x; 1.0405x over previous
"""Trainium2 Bass kernel for a character-CNN word encoder.

Computation (per word of W=20 chars):
  x = emb[chars]                       # [W, E=64] -> [E, W]
  y_k = conv1d(x, w_k, 'same') + b_k   # k in {1,3,5}, H=256 channels
  m_k = max_t relu(y_k)                # [H]
  out = concat(m1, m3, m5) @ lw.T + lb # [H]

Strategy (pure data parallel over N = B*S = 8192 words, 1024 words/core).
The original dma_gather embedding serialized ~190us/core on the Q7
descriptor generator; this version removes it entirely:
  - Embedding lookup as a MATMUL: the host uploads a one-hot encoding of
    the padded char stream (2 passes of 128 vocab rows each, bf16), and
    X[0:64, tok] = table.T @ onehot accumulates over the two K=128 passes
    into PSUM, then DVE-copies to SBUF bf16. Rows 64:128 of X (the
    one-column-left shift that lets one K=128 conv matmul contract two
    taps) are made by per-tile SBUF->SBUF DMAs.
  - Convs: tap-pair matmuls, 12 per 16-word block into 2 PSUM groups of
    3 regions x 512 fp32 (one bank each); PE streams back-to-back at
    ~135ns per 320-col matmul with LDWEIGHTS overlapped.
  - Drain: ACT copies each PSUM group block [128,3,16,20] into a bf16
    buffer [128,3,128,20]; the max over the 20 positions is a DVE
    tensor_tensor(max) tree (2x mode, ~2x cheaper per element than the
    1x tensor_reduce), with all slice starts 4-byte aligned. Tree/relu
    ops of chunk c are spread across chunk c+1's DVE stream, A/B groups
    interleaved, so dependent levels hide the DVE pipe drain and never
    block the PSUM-freeing drains. The last chunk's tree is split in w
    halves to shorten the epilogue.
  - relu(bias + max) via DVE tensor_scalar (add + max-0 fused).
  - Linear layer: 6 K=128 matmuls + K=1 ones-row matmul adding lb; its
    PSUM tile shares a 2-buffer pool with the embed tiles (8 banks total
    with the conv groups).
"""

import numpy as np
import ml_dtypes

import concourse.bass as bass
import concourse.tile as tile
import concourse.mybir as mybir
from concourse import bacc
from concourse.bass_utils import run_bass_kernel_spmd

BF16 = ml_dtypes.bfloat16
FP8 = ml_dtypes.float8_e4m3

# Problem shape (hardcoded per contest rules).
B, S, W = 64, 128, 20
VOCAB, E, H = 256, 64, 256
N_CORES = 8
NW = (B * S) // N_CORES       # words per core = 1024
WP = 22                       # word frame: [z z t0..t19]
PAD_TOK = 300                 # never matches vocab 0..255 -> all-zero one-hot col
CHUNK_W = 128                 # words per chunk
N_CHUNKS = NW // CHUNK_W      # 8
# Conv-matmul block sizes per chunk: 24-word blocks keep each PSUM region
# (nb*20 = 480 fp32) inside one bank while cutting the drain count per chunk
# from 16 to 12 (ACT fixed overhead) and the stall boundaries from 8 to 6.
BLOCKS = [24, 24, 24, 24, 24, 8]
BLOCK_OFF = [0, 24, 48, 72, 96, 120]
N_BLOCKS = len(BLOCKS)        # 6
TOK_CHUNK = CHUNK_W * WP      # 2816 tokens per chunk
HALO = 8                      # extra halo cols per chunk
TOKC = TOK_CHUNK + HALO       # 2824 cols per chunk tile
NT = NW * WP                  # 22528
NTP = NT + 16                 # padded one-hot cols in DRAM

# Embedding tile split of a chunk's TOKC columns (PSUM bank = 512 fp32).
EMB_TILES = [512, 512, 512, 512, 512, TOKC - 5 * 512]   # last = 264

# Conv tap-pair matmul plan (identical to v1).
# (region, conv_k, half, first_tap d, j0 = d - pad + 2, start, stop)
# Regions: 0=c5h0 1=c5h1 2=c3h0 3=c3h1 4=c1h0 5=c1h1.
def _mm_plan():
    plan_a, plan_b = [], []
    for half in (0, 1):
        r = half
        taps = [(0, 0), (2, 2), (4, 4)]
        for i, (d, j0) in enumerate(taps):
            plan_a.append((r, 5, half, d, j0, i == 0, i == len(taps) - 1))
    for half, r, dst in ((0, 2, plan_a), (1, 3, plan_b)):
        taps = [(0, 1), (2, 3)]
        for i, (d, j0) in enumerate(taps):
            dst.append((r, 3, half, d, j0, i == 0, i == len(taps) - 1))
    for half in (0, 1):
        r = 4 + half
        plan_b.append((r, 1, half, 0, 2, True, True))
    return plan_a, plan_b

PLAN_A, PLAN_B = _mm_plan()
WBLOCKS = PLAN_A + PLAN_B

# lw column ranges per region (reference concat order: conv1, conv3, conv5).
LW_COLS = {0: (512, 640), 1: (640, 768), 2: (256, 384), 3: (384, 512),
           4: (0, 128), 5: (128, 256)}

# ---- engine assignment (tuned from traces) ----
# ACT drains every (group, block) PSUM region into a bf16 buffer (the only
# PSUM-slot gating path); DVE runs the max tree at its 2x tensor_tensor
# mode on SBUF plus X-copies, relu, and the output-stage copy. Trees of
# chunk c-1 are spread between chunk c's blocks and the two groups' trees
# are interleaved so dependent levels hide the DVE pipe drain.


def _build_nc():
    f32 = mybir.dt.float32
    bf16 = mybir.dt.bfloat16
    ALU = mybir.AluOpType
    AF = mybir.ActivationFunctionType

    nc = bacc.Bacc("TRN2", target_bir_lowering=False, debug=False)

    fp8 = mybir.dt.float8e4
    # one-hot of the char stream as two fp8 k-tiles (vocab 0:128 / 128:256)
    oh_d = nc.dram_tensor("oh", [128, 2, NTP], fp8, kind="ExternalInput").ap()
    # embedding table split hi/lo in fp8: [:,0:2,:] = hi k-tiles, [:,2:4,:] =
    # lo k-tiles (lo = emb - fp8(emb), so X = hi@oh + lo@oh has ~0.4% error)
    tbl_d = nc.dram_tensor("tbl", [128, 4, 64], fp8, kind="ExternalInput").ap()
    wconv_d = nc.dram_tensor("wconv", [128, 12 * 128], bf16, kind="ExternalInput").ap()
    lwt_d = nc.dram_tensor("lwt", [128, 6 * 256], bf16, kind="ExternalInput").ap()
    cbias_d = nc.dram_tensor("cbias", [128, 6], f32, kind="ExternalInput").ap()
    lbias_d = nc.dram_tensor("lbias", [1, 256], bf16, kind="ExternalInput").ap()
    out_d = nc.dram_tensor("out", [NW, H], f32, kind="ExternalOutput").ap()

    with tile.TileContext(nc) as tc:
        with (
            tc.tile_pool(name="consts", bufs=1) as cpool,
            tc.tile_pool(name="oh", bufs=2) as ohpool,
            tc.tile_pool(name="xx", bufs=3) as xxpool,
            tc.tile_pool(name="dbuf", bufs=2) as dpool,
            tc.tile_pool(name="mtile", bufs=2) as mpool,
            tc.tile_pool(name="osb", bufs=2) as opool,
            tc.tile_pool(name="psC", bufs=2, space="PSUM") as psc_pool,
            # shared pool: embed tiles use [0:64, 0:512]; the linear-layer
            # output uses [:, 0:256]. bufs=2 double-buffers the embed tiles.
            tc.tile_pool(name="psE", bufs=2, space="PSUM") as pse_pool,
        ):
            # --- constants ---
            tbl_sb = cpool.tile([128, 4, 64], fp8, tag="tbl")
            nc.sync.dma_start(tbl_sb[:], tbl_d[:])
            wconv_sb = cpool.tile([128, 12 * 128], bf16, tag="wconv")
            nc.sync.dma_start(wconv_sb[:], wconv_d[:])
            lwt_sb = cpool.tile([128, 6 * 256], bf16, tag="lwt")
            nc.sync.dma_start(lwt_sb[:], lwt_d[:])
            cbias_sb = cpool.tile([128, 6], f32, tag="cbias")
            nc.sync.dma_start(cbias_sb[:], cbias_d[:])
            lbias_sb = cpool.tile([1, 256], bf16, tag="lbias")
            nc.sync.dma_start(lbias_sb[:], lbias_d[:])
            ones_sb = cpool.tile([1, 128], bf16, tag="ones")
            nc.vector.memset(ones_sb[:], 1.0)

            oh_tiles = {}     # c -> (oh0_t, oh1_t)
            xx_tiles = {}     # c -> xx tile
            dbufs = {}        # (c, g) -> drain buffer tile
            m_pres = {}       # c -> m_pre
            m_alls = {}       # c -> m_all

            def load_oh(c):
                oh_t = ohpool.tile([128, 2, TOKC], fp8, tag="oh")
                nc.sync.dma_start(oh_t[:], oh_d[:, :, c * TOK_CHUNK: c * TOK_CHUNK + TOKC])
                oh_tiles[c] = oh_t

            def embed_tile(c, t):
                # 2 fp8 DoubleRow matmuls (hi + lo correction, each K=256 via
                # two k-tiles at 0.5 cyc/col) into ps_e, then copy to xx bf16.
                if t == 0:
                    xx_t = xxpool.tile([128, TOKC], bf16, tag="xx")
                    xx_tiles[c] = xx_t
                oh_t = oh_tiles[c]
                c0 = sum(EMB_TILES[:t])
                n = EMB_TILES[t]
                ps_e = pse_pool.tile([128, 512], f32, tag="psE")
                nc.tensor.matmul(ps_e[0:64, 0:n], lhsT=tbl_sb[:, 0:2, :],
                                 rhs=oh_t[:, :, c0:c0 + n], start=True, stop=False,
                                 perf_mode=mybir.MatmulPerfMode.DoubleRow)
                nc.tensor.matmul(ps_e[0:64, 0:n], lhsT=tbl_sb[:, 2:4, :],
                                 rhs=oh_t[:, :, c0:c0 + n], start=False, stop=True,
                                 perf_mode=mybir.MatmulPerfMode.DoubleRow)
                xx = xx_tiles[c]
                nc.vector.tensor_copy(out=xx[0:64, c0:c0 + n], in_=ps_e[0:64, 0:n])

            def shift_part(c, t):
                # shifted-rows DMA for just the column range of embed tile t,
                # so conv blocks unblock tile-by-tile instead of waiting for
                # the whole chunk's X.
                c0 = sum(EMB_TILES[:t])
                n = EMB_TILES[t]
                a = c0 - 1 if t > 0 else 0
                e = c0 + n - 1
                xx = xx_tiles[c]
                nc.sync.dma_start(xx[64:128, a:e], xx[0:64, a + 1:e + 1])

            def conv_block(c, b):
                xx = xx_tiles[c]
                nb = BLOCKS[b]
                off = BLOCK_OFF[b]
                base = off * WP
                if b == 0:
                    dbuf_t = dpool.tile([128, 6, CHUNK_W, W], bf16, tag="dbuf")
                    dbufs[c] = dbuf_t

                def run_mms(plan, ps):
                    for (r, _k, _h, _d, j0, start, stop) in plan:
                        q = WBLOCKS.index((r, _k, _h, _d, j0, start, stop))
                        slot = r % 3
                        rhs = (
                            xx[:, base + j0: base + j0 + nb * WP]
                            .rearrange("p (w c) -> p w c", c=WP)[:, :, 0:W]
                        )
                        nc.tensor.matmul(
                            ps[:, slot * 512: slot * 512 + nb * W],
                            lhsT=wconv_sb[:, q * 128:(q + 1) * 128],
                            rhs=rhs, start=start, stop=stop,
                        )

                def drain(g, ps):
                    pv = (
                        ps[:, 0:1536]
                        .rearrange("p (r s) -> p r s", s=512)[:, :, 0:nb * W]
                        .rearrange("p r (w c) -> p r w c", c=W)
                    )
                    dst = dbufs[c][:, g * 3:(g + 1) * 3, off:off + nb, :]
                    nc.scalar.copy(out=dst, in_=pv)

                ps_a = psc_pool.tile([128, 1536], f32, tag="psC")
                run_mms(PLAN_A, ps_a)
                ps_b = psc_pool.tile([128, 1536], f32, tag="psC")
                run_mms(PLAN_B, ps_b)
                drain(0, ps_a)
                drain(1, ps_b)

            def tree_ops(c, w0, w1):
                # Max over the W=20 position cols of the (merged A+B) dbuf
                # for word range [w0, w1), writing m_pre rows. One op per
                # level spanning all 6 regions. The first half-chunk's tree
                # runs inside its own chunk (drains of blocks 0-2 cover w
                # 0:72), so only the second half's serial chain crosses the
                # chunk boundary. All slice starts stay 4-byte aligned (2x).
                MAX = mybir.AluOpType.max
                ops = []

                def lvl(s0, s1, n):
                    d = dbufs[c][:, :, w0:w1, :]
                    ops.append(lambda d=d, s0=s0, s1=s1, n=n: nc.vector.tensor_tensor(
                        out=d[:, :, :, s0:s0 + n], in0=d[:, :, :, s0:s0 + n],
                        in1=d[:, :, :, s1:s1 + n], op=MAX))

                lvl(0, 8, 8)      # P1: t[0:8]  = max(t[0:8],  t[8:16])
                lvl(0, 4, 4)      # P2: t[0:4]  = max(t[0:4],  t[4:8])
                lvl(16, 18, 2)    # P3: t[16:18] = max(t[16:18], t[18:20])
                lvl(0, 2, 2)      # P4: t[0:2]  = max(t[0:2],  t[2:4])
                lvl(0, 16, 2)     # P5: t[0:2]  = max(t[0:2],  t[16:18])

                def p6():         # P6: m_pre rows = max(t[0], t[1])
                    d = dbufs[c][:, :, w0:w1, :]
                    mp = m_pres[c][:, :, w0:w1]
                    t0 = d[:, :, :, 0:1].rearrange("p r w c -> p r (w c)")
                    t1 = d[:, :, :, 1:2].rearrange("p r w c -> p r (w c)")
                    nc.vector.tensor_tensor(out=mp, in0=t0, in1=t1, op=MAX)
                ops.append(p6)
                return ops

            def relu_ops(c):
                ops = []
                for r in range(6):       # relu(bias + m_pre) -> m_all
                    def rl(r=r):
                        nc.vector.tensor_scalar(
                            out=m_alls[c][:, r, :], in0=m_pres[c][:, r, :],
                            scalar1=cbias_sb[:, r:r + 1], scalar2=0.0,
                            op0=mybir.AluOpType.add, op1=mybir.AluOpType.max)
                    ops.append(rl)
                return ops

            def linear(c):
                ps_t = pse_pool.tile([128, 512], f32, tag="psE")
                op = ps_t[:, 0:256]
                for r in range(6):
                    nc.tensor.matmul(
                        op[:], lhsT=m_alls[c][:, r, :],
                        rhs=lwt_sb[:, r * 256:(r + 1) * 256],
                        start=(r == 0), stop=False,
                    )
                nc.tensor.matmul(op[:], lhsT=ones_sb[0:1, :], rhs=lbias_sb[0:1, :],
                                 start=False, stop=True)
                osb = opool.tile([128, 256], f32, tag="osb")
                nc.vector.tensor_copy(out=osb[:], in_=op[:])
                nc.sync.dma_start(out_d[c * CHUNK_W:(c + 1) * CHUNK_W, :], osb[:])

            # --- schedule ---
            load_oh(0)
            load_oh(1)

            # embed tiles needed (exclusive upper idx) before conv block b:
            # block b reads xx cols up to (off+nb-1)*22+23 (+1 shift source).
            TILES_FOR_BLOCK = [2, 3, 4, 5, 6, 6]
            emb_done = {}

            def ensure_embed(c, upto):
                while emb_done.get(c, 0) < min(upto, len(EMB_TILES)):
                    t = emb_done.get(c, 0)
                    embed_tile(c, t)
                    shift_part(c, t)
                    emb_done[c] = t + 1

            pending = []  # DVE tree/relu thunks of the previous chunk
            for c in range(N_CHUNKS):
                if c + 2 < N_CHUNKS:
                    load_oh(c + 2)
                m_pre = mpool.tile([128, 6, CHUNK_W], bf16, tag="m_pre")
                m_all = mpool.tile([128, 6, CHUNK_W], bf16, tag="m_all")
                m_pres[c] = m_pre
                m_alls[c] = m_all
                for b in range(N_BLOCKS):
                    ensure_embed(c, TILES_FOR_BLOCK[b])
                    if c + 1 < N_CHUNKS:
                        ensure_embed(c + 1, b + 1)
                    conv_block(c, b)
                    # spread queued DVE ops: [half2(c-1) tree, relus(c-1),
                    # half1(c) tree]; 3/block puts the relus before b==4
                    for _ in range(3):
                        if pending:
                            pending.pop(0)()
                    if b == 4 and c >= 1:
                        linear(c - 1)
                    if b == 2:
                        # blocks 0-2 (w 0:72) are drained: this chunk's first
                        # half-tree can start mid-chunk
                        pending += tree_ops(c, 0, 72)
                while pending:
                    pending.pop(0)()
                pending = tree_ops(c, 72, CHUNK_W) + relu_ops(c)
            while pending:
                pending.pop(0)()
            linear(N_CHUNKS - 1)

    nc.compile()
    return nc


def _prep_maps(chars, emb, w1, b1, w3, b3, w5, b5, lw, lb):
    flat = np.asarray(chars).reshape(-1, W).astype(np.int64)  # [8192, 20]
    emb = np.asarray(emb, dtype=np.float32)
    lw = np.asarray(lw, dtype=np.float32)
    convs = {1: np.asarray(w1, np.float32), 3: np.asarray(w3, np.float32),
             5: np.asarray(w5, np.float32)}
    biases = {1: np.asarray(b1, np.float32), 3: np.asarray(b3, np.float32),
              5: np.asarray(b5, np.float32)}

    # tbl: fp8 hi/lo split, k-tile layout [128, 4, 64]:
    # [:,0,:]=hi vocab 0:128, [:,1,:]=hi vocab 128:256, [:,2:4,:]=lo parts.
    hi = emb.astype(FP8)
    lo = (emb - hi.astype(np.float32)).astype(FP8)
    tbl = np.zeros((128, 4, 64), dtype=FP8)
    tbl[:, 0, :] = hi[0:128, :]
    tbl[:, 1, :] = hi[128:256, :]
    tbl[:, 2, :] = lo[0:128, :]
    tbl[:, 3, :] = lo[128:256, :]

    wconv = np.zeros((128, 12 * 128), dtype=BF16)
    for q, (r, k, half, d, j0, _s, _e) in enumerate(WBLOCKS):
        wk = convs[k]  # [H, E, k]
        blk = np.zeros((128, 128), dtype=np.float32)
        blk[:E, :] = wk[half * 128:(half + 1) * 128, :, d].T
        if d + 1 < k:
            blk[E:, :] = wk[half * 128:(half + 1) * 128, :, d + 1].T
        wconv[:, q * 128:(q + 1) * 128] = blk.astype(BF16)

    lwt = np.zeros((128, 6 * 256), dtype=BF16)
    for r in range(6):
        lo, hi = LW_COLS[r]
        lwt[:, r * 256:(r + 1) * 256] = lw[:, lo:hi].T.astype(BF16)

    cbias = np.zeros((128, 6), dtype=np.float32)
    for r, (k, half) in enumerate([(5, 0), (5, 1), (3, 0), (3, 1), (1, 0), (1, 1)]):
        cbias[:, r] = biases[k][half * 128:(half + 1) * 128]

    lbias = np.asarray(lb, np.float32).reshape(1, 256).astype(BF16)

    rlo = np.arange(128, dtype=np.int16)[:, None]
    in_maps = []
    for c in range(N_CORES):
        words = flat[c * NW:(c + 1) * NW]  # [NW, 20]
        padded = np.full((NW, WP), PAD_TOK, dtype=np.int16)
        padded[:, 2:2 + W] = words
        stream = np.full(NTP, PAD_TOK, dtype=np.int16)
        stream[:NT] = padded.reshape(-1)
        oh = np.zeros((128, 2, NTP), dtype=FP8)
        oh[:, 0, :] = (stream[None, :] == rlo)
        oh[:, 1, :] = (stream[None, :] == (rlo + 128))
        in_maps.append({
            "oh": oh, "tbl": tbl, "wconv": wconv, "lwt": lwt,
            "cbias": cbias, "lbias": lbias,
        })
    return in_maps


_NC_CACHE = {}


def run(inputs, trace=False):
    if "nc" not in _NC_CACHE:
        _NC_CACHE["nc"] = _build_nc()
    nc = _NC_CACHE["nc"]
    in_maps = _prep_maps(**inputs)
    res = run_bass_kernel_spmd(nc, in_maps, list(range(N_CORES)), trace=trace)
    out = np.concatenate([res.results[i]["out"] for i in range(N_CORES)], axis=0)
    return out.reshape(B, S, H).astype(np.float32), res


def kernel(**inputs):
    out, _ = run(inputs)
    return out


# revision 7
# speedup vs baseline: 1.1820x; 1.0079x over previous
"""Trainium2 Bass kernel for a character-CNN word encoder.

Computation (per word of W=20 chars):
  x = emb[chars]                       # [W, E=64] -> [E, W]
  y_k = conv1d(x, w_k, 'same') + b_k   # k in {1,3,5}, H=256 channels
  m_k = max_t relu(y_k)                # [H]
  out = concat(m1, m3, m5) @ lw.T + lb # [H]

Strategy (pure data parallel over N = B*S = 8192 words, 1024 words/core).
The original dma_gather embedding serialized ~190us/core on the Q7
descriptor generator; this version removes it entirely:
  - Embedding lookup as a MATMUL: the host uploads a one-hot encoding of
    the padded char stream (2 passes of 128 vocab rows each, bf16), and
    X[0:64, tok] = table.T @ onehot accumulates over the two K=128 passes
    into PSUM, then DVE-copies to SBUF bf16. Rows 64:128 of X (the
    one-column-left shift that lets one K=128 conv matmul contract two
    taps) are made by per-tile SBUF->SBUF DMAs.
  - Convs: tap-pair matmuls, 12 per 16-word block into 2 PSUM groups of
    3 regions x 512 fp32 (one bank each); PE streams back-to-back at
    ~135ns per 320-col matmul with LDWEIGHTS overlapped.
  - Drain: ACT copies each PSUM group block [128,3,16,20] into a bf16
    buffer [128,3,128,20]; the max over the 20 positions is a DVE
    tensor_tensor(max) tree (2x mode, ~2x cheaper per element than the
    1x tensor_reduce), with all slice starts 4-byte aligned. Tree/relu
    ops of chunk c are spread across chunk c+1's DVE stream, A/B groups
    interleaved, so dependent levels hide the DVE pipe drain and never
    block the PSUM-freeing drains. The last chunk's tree is split in w
    halves to shorten the epilogue.
  - relu(bias + max) via DVE tensor_scalar (add + max-0 fused).
  - Linear layer: 6 K=128 matmuls + K=1 ones-row matmul adding lb; its
    PSUM tile shares a 2-buffer pool with the embed tiles (8 banks total
    with the conv groups).
"""

import numpy as np
import ml_dtypes

import concourse.bass as bass
import concourse.tile as tile
import concourse.mybir as mybir
from concourse import bacc
from concourse.bass_utils import run_bass_kernel_spmd

BF16 = ml_dtypes.bfloat16
FP8 = ml_dtypes.float8_e4m3

# Problem shape (hardcoded per contest rules).
B, S, W = 64, 128, 20
VOCAB, E, H = 256, 64, 256
N_CORES = 8
NW = (B * S) // N_CORES       # words per core = 1024
WP = 22                       # word frame: [z z t0..t19]
PAD_TOK = 300                 # never matches vocab 0..255 -> all-zero one-hot col
CHUNK_W = 128                 # words per chunk
N_CHUNKS = NW // CHUNK_W      # 8
# Conv-matmul block sizes per chunk: 24-word blocks keep each PSUM region
# (nb*20 = 480 fp32) inside one bank while cutting the drain count per chunk
# from 16 to 12 (ACT fixed overhead) and the stall boundaries from 8 to 6.
BLOCKS = [24, 24, 24, 24, 24, 8]
BLOCK_OFF = [0, 24, 48, 72, 96, 120]
N_BLOCKS = len(BLOCKS)        # 6
TOK_CHUNK = CHUNK_W * WP      # 2816 tokens per chunk
HALO = 8                      # extra halo cols per chunk
TOKC = TOK_CHUNK + HALO       # 2824 cols per chunk tile
NT = NW * WP                  # 22528
NTP = NT + 16                 # padded one-hot cols in DRAM

# Embedding tile split of a chunk's TOKC columns (PSUM bank = 512 fp32).
EMB_TILES = [512, 512, 512, 512, 512, TOKC - 5 * 512]   # last = 264

# Conv tap-pair matmul plan (identical to v1).
# (region, conv_k, half, first_tap d, j0 = d - pad + 2, start, stop)
# Regions: 0=c5h0 1=c5h1 2=c3h0 3=c3h1 4=c1h0 5=c1h1.
def _mm_plan():
    plan_a, plan_b = [], []
    for half in (0, 1):
        r = half
        taps = [(0, 0), (2, 2), (4, 4)]
        for i, (d, j0) in enumerate(taps):
            plan_a.append((r, 5, half, d, j0, i == 0, i == len(taps) - 1))
    for half, r, dst in ((0, 2, plan_a), (1, 3, plan_b)):
        taps = [(0, 1), (2, 3)]
        for i, (d, j0) in enumerate(taps):
            dst.append((r, 3, half, d, j0, i == 0, i == len(taps) - 1))
    for half in (0, 1):
        r = 4 + half
        plan_b.append((r, 1, half, 0, 2, True, True))
    return plan_a, plan_b

PLAN_A, PLAN_B = _mm_plan()
WBLOCKS = PLAN_A + PLAN_B

# lw column ranges per region (reference concat order: conv1, conv3, conv5).
LW_COLS = {0: (512, 640), 1: (640, 768), 2: (256, 384), 3: (384, 512),
           4: (0, 128), 5: (128, 256)}

# ---- engine assignment (tuned from traces) ----
# ACT drains every (group, block) PSUM region into a bf16 buffer (the only
# PSUM-slot gating path); DVE runs the max tree at its 2x tensor_tensor
# mode on SBUF plus X-copies, relu, and the output-stage copy. Trees of
# chunk c-1 are spread between chunk c's blocks and the two groups' trees
# are interleaved so dependent levels hide the DVE pipe drain.


def _build_nc():
    f32 = mybir.dt.float32
    bf16 = mybir.dt.bfloat16
    ALU = mybir.AluOpType
    AF = mybir.ActivationFunctionType

    nc = bacc.Bacc("TRN2", target_bir_lowering=False, debug=False)

    fp8 = mybir.dt.float8e4
    # one-hot of the char stream as two fp8 k-tiles (vocab 0:128 / 128:256)
    oh_d = nc.dram_tensor("oh", [128, 2, NTP], fp8, kind="ExternalInput").ap()
    # embedding table split hi/lo in fp8: [:,0:2,:] = hi k-tiles, [:,2:4,:] =
    # lo k-tiles (lo = emb - fp8(emb), so X = hi@oh + lo@oh has ~0.4% error)
    tbl_d = nc.dram_tensor("tbl", [128, 4, 64], fp8, kind="ExternalInput").ap()
    wconv_d = nc.dram_tensor("wconv", [128, 12 * 128], bf16, kind="ExternalInput").ap()
    lwt_d = nc.dram_tensor("lwt", [128, 6 * 256], bf16, kind="ExternalInput").ap()
    cbias_d = nc.dram_tensor("cbias", [128, 6], f32, kind="ExternalInput").ap()
    lbias_d = nc.dram_tensor("lbias", [1, 256], bf16, kind="ExternalInput").ap()
    out_d = nc.dram_tensor("out", [NW, H], f32, kind="ExternalOutput").ap()

    with tile.TileContext(nc) as tc:
        with (
            tc.tile_pool(name="consts", bufs=1) as cpool,
            tc.tile_pool(name="oh", bufs=2) as ohpool,
            tc.tile_pool(name="xx", bufs=3) as xxpool,
            tc.tile_pool(name="dbuf", bufs=2) as dpool,
            tc.tile_pool(name="mtile", bufs=2) as mpool,
            tc.tile_pool(name="osb", bufs=2) as opool,
            tc.tile_pool(name="psC", bufs=2, space="PSUM") as psc_pool,
            # shared pool: embed tiles use [0:64, 0:512]; the linear-layer
            # output uses [:, 0:256]. bufs=2 double-buffers the embed tiles.
            tc.tile_pool(name="psE", bufs=2, space="PSUM") as pse_pool,
        ):
            # --- constants ---
            tbl_sb = cpool.tile([128, 4, 64], fp8, tag="tbl")
            nc.sync.dma_start(tbl_sb[:], tbl_d[:])
            wconv_sb = cpool.tile([128, 12 * 128], bf16, tag="wconv")
            nc.sync.dma_start(wconv_sb[:], wconv_d[:])
            lwt_sb = cpool.tile([128, 6 * 256], bf16, tag="lwt")
            nc.sync.dma_start(lwt_sb[:], lwt_d[:])
            cbias_sb = cpool.tile([128, 6], f32, tag="cbias")
            nc.sync.dma_start(cbias_sb[:], cbias_d[:])
            lbias_sb = cpool.tile([1, 256], bf16, tag="lbias")
            nc.sync.dma_start(lbias_sb[:], lbias_d[:])
            ones_sb = cpool.tile([1, 128], bf16, tag="ones")
            nc.vector.memset(ones_sb[:], 1.0)

            oh_tiles = {}     # c -> (oh0_t, oh1_t)
            xx_tiles = {}     # c -> xx tile
            dbufs = {}        # (c, g) -> drain buffer tile
            m_pres = {}       # c -> m_pre
            m_alls = {}       # c -> m_all

            def load_oh(c):
                oh_t = ohpool.tile([128, 2, TOKC], fp8, tag="oh")
                nc.sync.dma_start(oh_t[:], oh_d[:, :, c * TOK_CHUNK: c * TOK_CHUNK + TOKC])
                oh_tiles[c] = oh_t

            def embed_tile(c, t):
                # 2 fp8 DoubleRow matmuls (hi + lo correction, each K=256 via
                # two k-tiles at 0.5 cyc/col) into ps_e, then copy to xx bf16.
                if t == 0:
                    xx_t = xxpool.tile([128, TOKC], bf16, tag="xx")
                    xx_tiles[c] = xx_t
                oh_t = oh_tiles[c]
                c0 = sum(EMB_TILES[:t])
                n = EMB_TILES[t]
                ps_e = pse_pool.tile([128, 512], f32, tag="psE")
                nc.tensor.matmul(ps_e[0:64, 0:n], lhsT=tbl_sb[:, 0:2, :],
                                 rhs=oh_t[:, :, c0:c0 + n], start=True, stop=False,
                                 perf_mode=mybir.MatmulPerfMode.DoubleRow)
                nc.tensor.matmul(ps_e[0:64, 0:n], lhsT=tbl_sb[:, 2:4, :],
                                 rhs=oh_t[:, :, c0:c0 + n], start=False, stop=True,
                                 perf_mode=mybir.MatmulPerfMode.DoubleRow)
                xx = xx_tiles[c]
                nc.vector.tensor_copy(out=xx[0:64, c0:c0 + n], in_=ps_e[0:64, 0:n])

            def shift_part(c, t):
                # shifted-rows DMA for just the column range of embed tile t,
                # so conv blocks unblock tile-by-tile instead of waiting for
                # the whole chunk's X.
                c0 = sum(EMB_TILES[:t])
                n = EMB_TILES[t]
                a = c0 - 1 if t > 0 else 0
                e = c0 + n - 1
                xx = xx_tiles[c]
                nc.sync.dma_start(xx[64:128, a:e], xx[0:64, a + 1:e + 1])

            def conv_block(c, b):
                xx = xx_tiles[c]
                nb = BLOCKS[b]
                off = BLOCK_OFF[b]
                base = off * WP
                if b == 0:
                    dbuf_t = dpool.tile([128, 6, CHUNK_W, W], bf16, tag="dbuf")
                    dbufs[c] = dbuf_t

                def run_mms(plan, ps):
                    for (r, _k, _h, _d, j0, start, stop) in plan:
                        q = WBLOCKS.index((r, _k, _h, _d, j0, start, stop))
                        slot = r % 3
                        rhs = (
                            xx[:, base + j0: base + j0 + nb * WP]
                            .rearrange("p (w c) -> p w c", c=WP)[:, :, 0:W]
                        )
                        nc.tensor.matmul(
                            ps[:, slot * 512: slot * 512 + nb * W],
                            lhsT=wconv_sb[:, q * 128:(q + 1) * 128],
                            rhs=rhs, start=start, stop=stop,
                        )

                def drain(g, ps):
                    pv = (
                        ps[:, 0:1536]
                        .rearrange("p (r s) -> p r s", s=512)[:, :, 0:nb * W]
                        .rearrange("p r (w c) -> p r w c", c=W)
                    )
                    dst = dbufs[c][:, g * 3:(g + 1) * 3, off:off + nb, :]
                    nc.scalar.copy(out=dst, in_=pv)

                ps_a = psc_pool.tile([128, 1536], f32, tag="psC")
                run_mms(PLAN_A, ps_a)
                ps_b = psc_pool.tile([128, 1536], f32, tag="psC")
                run_mms(PLAN_B, ps_b)
                drain(0, ps_a)
                drain(1, ps_b)

            def tree_ops(c, w0, w1):
                # Max over the W=20 position cols of the (merged A+B) dbuf
                # for word range [w0, w1), writing m_pre rows. One op per
                # level spanning all 6 regions. The first half-chunk's tree
                # runs inside its own chunk (drains of blocks 0-2 cover w
                # 0:72), so only the second half's serial chain crosses the
                # chunk boundary. All slice starts stay 4-byte aligned (2x).
                MAX = mybir.AluOpType.max
                ops = []

                def lvl(s0, s1, n):
                    d = dbufs[c][:, :, w0:w1, :]
                    ops.append(lambda d=d, s0=s0, s1=s1, n=n: nc.vector.tensor_tensor(
                        out=d[:, :, :, s0:s0 + n], in0=d[:, :, :, s0:s0 + n],
                        in1=d[:, :, :, s1:s1 + n], op=MAX))

                lvl(0, 8, 8)      # P1: t[0:8]  = max(t[0:8],  t[8:16])
                lvl(0, 4, 4)      # P2: t[0:4]  = max(t[0:4],  t[4:8])
                lvl(16, 18, 2)    # P3: t[16:18] = max(t[16:18], t[18:20])
                lvl(0, 2, 2)      # P4: t[0:2]  = max(t[0:2],  t[2:4])
                lvl(0, 16, 2)     # P5: t[0:2]  = max(t[0:2],  t[16:18])

                def p6():         # P6: m_pre rows = max(t[0], t[1])
                    d = dbufs[c][:, :, w0:w1, :]
                    mp = m_pres[c][:, :, w0:w1]
                    t0 = d[:, :, :, 0:1].rearrange("p r w c -> p r (w c)")
                    t1 = d[:, :, :, 1:2].rearrange("p r w c -> p r (w c)")
                    nc.vector.tensor_tensor(out=mp, in0=t0, in1=t1, op=MAX)
                ops.append(p6)
                return ops

            def relu_ops(c):
                ops = []
                for r in range(6):       # relu(bias + m_pre) -> m_all
                    def rl(r=r):
                        nc.vector.tensor_scalar(
                            out=m_alls[c][:, r, :], in0=m_pres[c][:, r, :],
                            scalar1=cbias_sb[:, r:r + 1], scalar2=0.0,
                            op0=mybir.AluOpType.add, op1=mybir.AluOpType.max)
                    ops.append(rl)
                return ops

            def linear(c):
                ps_t = pse_pool.tile([128, 512], f32, tag="psE")
                op = ps_t[:, 0:256]
                for r in range(6):
                    nc.tensor.matmul(
                        op[:], lhsT=m_alls[c][:, r, :],
                        rhs=lwt_sb[:, r * 256:(r + 1) * 256],
                        start=(r == 0), stop=False,
                    )
                nc.tensor.matmul(op[:], lhsT=ones_sb[0:1, :], rhs=lbias_sb[0:1, :],
                                 start=False, stop=True)
                osb = opool.tile([128, 256], f32, tag="osb")
                nc.vector.tensor_copy(out=osb[:], in_=op[:])
                nc.sync.dma_start(out_d[c * CHUNK_W:(c + 1) * CHUNK_W, :], osb[:])

            # --- schedule ---
            load_oh(0)
            load_oh(1)

            # embed tiles needed (exclusive upper idx) before conv block b:
            # block b reads xx cols up to (off+nb-1)*22+23 (+1 shift source).
            TILES_FOR_BLOCK = [2, 3, 4, 5, 6, 6]
            emb_done = {}

            def ensure_embed(c, upto):
                while emb_done.get(c, 0) < min(upto, len(EMB_TILES)):
                    t = emb_done.get(c, 0)
                    embed_tile(c, t)
                    shift_part(c, t)
                    emb_done[c] = t + 1

            pending = []  # DVE tree/relu thunks of the previous chunk
            for c in range(N_CHUNKS):
                if c + 2 < N_CHUNKS:
                    load_oh(c + 2)
                m_pre = mpool.tile([128, 6, CHUNK_W], bf16, tag="m_pre")
                m_all = mpool.tile([128, 6, CHUNK_W], bf16, tag="m_all")
                m_pres[c] = m_pre
                m_alls[c] = m_all
                for b in range(N_BLOCKS):
                    ensure_embed(c, TILES_FOR_BLOCK[b])
                    if c + 1 < N_CHUNKS:
                        ensure_embed(c + 1, b + 1)
                    conv_block(c, b)
                    # spread queued DVE ops: [third3(c-1), relus(c-1),
                    # third1(c), third2(c)]; 4/block finishes the relus by
                    # b==2, well before linear(c-1) at b==4
                    for _ in range(4):
                        if pending:
                            pending.pop(0)()
                    if b == 4 and c >= 1:
                        linear(c - 1)
                    if b == 1:
                        # blocks 0-1 (w 0:48) drained: first third-tree
                        pending += tree_ops(c, 0, 48)
                    if b == 3:
                        # blocks 2-3 (w 48:96) drained: second third-tree
                        pending += tree_ops(c, 48, 96)
                while pending:
                    pending.pop(0)()
                pending = tree_ops(c, 96, CHUNK_W) + relu_ops(c)
            while pending:
                pending.pop(0)()
            linear(N_CHUNKS - 1)

    nc.compile()
    return nc


def _prep_maps(chars, emb, w1, b1, w3, b3, w5, b5, lw, lb):
    flat = np.asarray(chars).reshape(-1, W).astype(np.int64)  # [8192, 20]
    emb = np.asarray(emb, dtype=np.float32)
    lw = np.asarray(lw, dtype=np.float32)
    convs = {1: np.asarray(w1, np.float32), 3: np.asarray(w3, np.float32),
             5: np.asarray(w5, np.float32)}
    biases = {1: np.asarray(b1, np.float32), 3: np.asarray(b3, np.float32),
              5: np.asarray(b5, np.float32)}

    # tbl: fp8 hi/lo split, k-tile layout [128, 4, 64]:
    # [:,0,:]=hi vocab 0:128, [:,1,:]=hi vocab 128:256, [:,2:4,:]=lo parts.
    hi = emb.astype(FP8)
    lo = (emb - hi.astype(np.float32)).astype(FP8)
    tbl = np.zeros((128, 4, 64), dtype=FP8)
    tbl[:, 0, :] = hi[0:128, :]
    tbl[:, 1, :] = hi[128:256, :]
    tbl[:, 2, :] = lo[0:128, :]
    tbl[:, 3, :] = lo[128:256, :]

    wconv = np.zeros((128, 12 * 128), dtype=BF16)
    for q, (r, k, half, d, j0, _s, _e) in enumerate(WBLOCKS):
        wk = convs[k]  # [H, E, k]
        blk = np.zeros((128, 128), dtype=np.float32)
        blk[:E, :] = wk[half * 128:(half + 1) * 128, :, d].T
        if d + 1 < k:
            blk[E:, :] = wk[half * 128:(half + 1) * 128, :, d + 1].T
        wconv[:, q * 128:(q + 1) * 128] = blk.astype(BF16)

    lwt = np.zeros((128, 6 * 256), dtype=BF16)
    for r in range(6):
        lo, hi = LW_COLS[r]
        lwt[:, r * 256:(r + 1) * 256] = lw[:, lo:hi].T.astype(BF16)

    cbias = np.zeros((128, 6), dtype=np.float32)
    for r, (k, half) in enumerate([(5, 0), (5, 1), (3, 0), (3, 1), (1, 0), (1, 1)]):
        cbias[:, r] = biases[k][half * 128:(half + 1) * 128]

    lbias = np.asarray(lb, np.float32).reshape(1, 256).astype(BF16)

    rlo = np.arange(128, dtype=np.int16)[:, None]
    in_maps = []
    for c in range(N_CORES):
        words = flat[c * NW:(c + 1) * NW]  # [NW, 20]
        padded = np.full((NW, WP), PAD_TOK, dtype=np.int16)
        padded[:, 2:2 + W] = words
        stream = np.full(NTP, PAD_TOK, dtype=np.int16)
        stream[:NT] = padded.reshape(-1)
        oh = np.zeros((128, 2, NTP), dtype=FP8)
        oh[:, 0, :] = (stream[None, :] == rlo)
        oh[:, 1, :] = (stream[None, :] == (rlo + 128))
        in_maps.append({
            "oh": oh, "tbl": tbl, "wconv": wconv, "lwt": lwt,
            "cbias": cbias, "lbias": lbias,
        })
    return in_maps


_NC_CACHE = {}


def run(inputs, trace=False):
    if "nc" not in _NC_CACHE:
        _NC_CACHE["nc"] = _build_nc()
    nc = _NC_CACHE["nc"]
    in_maps = _prep_maps(**inputs)
    res = run_bass_kernel_spmd(nc, in_maps, list(range(N_CORES)), trace=trace)
    out = np.concatenate([res.results[i]["out"] for i in range(N_CORES)], axis=0)
    return out.reshape(B, S, H).astype(np.float32), res


def kernel(**inputs):
    out, _ = run(inputs)
    return out
